# revision 29
# baseline (speedup 1.0000x reference)
"""Causal self-attention (B=4, T=2048, C=1024, H=16) on 8 TRN2 NeuronCores.

Sharding: data-parallel on batch (4) x tensor-parallel on heads (2 groups of
8). Core i handles batch i//2 and head-group i%2.

Schedule: j-OUTER rounds — for each q-tile j (512 wide), all 4 head-pairs
attend in sequence. This staggers the pairwise AllGathers across the whole
kernel instead of stacking them at the tail (the prior pair-outer schedule
spent its last ~50us at half HAM clock waiting on the final pair's AG chain).

Per (pair, j) tile: scores^T [k, q] stripes, 2-deep pipelined st -> exp ->
av; causal diagonal blocks N-trimmed + 128x128 triangle masked by DVE mul
with a 0/1 mask. exp is safe without max subtraction (scores*hs^-0.5 are
O(+-10)). sum(exp) rides as a ones column in v (av out M=65). The two
heads' score matmuls land on PE row tiles (0,0)/(64,0) and run concurrent.

Normalize: per head the exact 4-op chain (stage row to base-0 SBUF, fast
reciprocal, gpsimd partition_broadcast, mul) — do not add ops; a 5-op chain
collapses the HW clock. rr=0 mul writes y2[0:64], rr=1 writes y2[64:128]
(partition-shifted DVE write, HW-validated), then ONE dma lands y2 in the
AG input segment.

Collectives: 12 pairwise AllGathers — per pair one T-half segment for
j=0/1 and quarter segments for j=2, j=3. After each AG, yr tiles ([128,512]
per (pair, j, group), 1KB DMA lines) are fetched to SBUF; projection for
t-chunk t accumulates 8 matmuls (4 pairs x 2 groups) into one PSUM bank,
one DVE add folds b_proj, out DMA rotates across queues. Output lands
spread over the second half of the kernel instead of all at the end.

Startup: input DMAs priority-ordered across 5 trigger queues (sync, gpsimd,
scalar, vector, tensor): first-matmul deps (bqk, wqk[0]/wqk[4] halves,
x quarter 0) first, then wv, then the rest in consumption order.

dtypes: matmul operands bf16, accumulation fp32 in PSUM, softmax
normalization fp32. (fp8 anywhere fails the 2e-2 gate: ~6% y error.)

HW gotchas (CoreSim passes these; only real HW fails):
  - a single 65-partition DVE copy from PSUM silently corrupts data
  - gpsimd custom-DVE reciprocal misreads PSUM and misreads inputs at a
    nonzero base partition (stage rows into base-0 SBUF first)
  - junk "heater" matmuls and 16x fine-grained AllGathers both regress
"""

import os
import sys
from contextlib import ExitStack

import numpy as np
import ml_dtypes

if "/opt/trn_rl_repo" not in sys.path:
    sys.path.insert(0, "/opt/trn_rl_repo")

import concourse.bass as bass
import concourse.mybir as mybir
import concourse.tile as tile
from concourse import bacc
from concourse import bass_utils

F32 = mybir.dt.float32
BF16 = mybir.dt.bfloat16
P = 128          # SBUF partitions
QT = 512         # q tile (matmul free dim)
KC = 128         # k chunk (psum partition dim)
HS = 64          # head size
KPQ = QT // KC   # k chunks per q tile

N_CORES = 8
PAIRS = [[0, 1], [2, 3], [4, 5], [6, 7]]

B_FULL, T_FULL, C_FULL, H_FULL = 4, 2048, 1024, 16


def build_nc(T=T_FULL, C=C_FULL, HL=H_FULL // 2):
    """Build the SPMD graph for one core (all 8 cores run the same graph).

    Per-core input tensors:
      xT    [C, T] bf16       x[b] transposed
      wqk   [2CL/P, P, C/P, P] bf16  w_attn q|k cols, host-shuffled [f,p,c,m]
      wv_s  [P, C/P, CL] bf16 w_attn v cols, host-shuffled [p,c,m]
      wp_s  [P, C/P, CL] bf16 w_proj (all 1024 rows as 2 groups x 4 pairs,
                              this core's 512 out cols), host-shuffled
      bqk   [P, 2*CL/P] f32, bv [CL] f32, bp [CL] f32
    Output: out [T, CL] f32.
    """
    CL = HL * HS                 # local width (q, k, v, out-cols each)
    n_cc = C // P                # x feature chunks (8)
    n_f = 2 * CL // P            # q|k f-tiles (4 q then 4 k)
    n_jt = T // QT               # q tiles / rounds (4)
    n_kt = T // KC               # k chunks / v t-chunks (16)
    n_pair = HL // 2             # head pairs (4)
    TH = T // 2
    scale = HS ** -0.5

    nc = bacc.Bacc("TRN2", target_bir_lowering=False, debug=False,
                   num_devices=N_CORES)

    xT = nc.dram_tensor("xT", [C, T], BF16, kind="ExternalInput").ap()
    wqk = nc.dram_tensor("wqk", [n_f, P, n_cc, P], BF16,
                         kind="ExternalInput").ap()
    wv_s = nc.dram_tensor("wv_s", [P, n_cc, CL], BF16,
                          kind="ExternalInput").ap()
    wp_s = nc.dram_tensor("wp_s", [P, n_cc, CL], BF16,
                          kind="ExternalInput").ap()
    bqk = nc.dram_tensor("bqk", [P, 2 * CL // P], F32,
                         kind="ExternalInput").ap()
    bv = nc.dram_tensor("bv", [CL], F32, kind="ExternalInput").ap()
    bp = nc.dram_tensor("bp", [CL], F32, kind="ExternalInput").ap()
    out_ext = nc.dram_tensor("out", [T, CL], F32, kind="ExternalOutput").ap()

    with ExitStack() as ctx:
        tc = ctx.enter_context(tile.TileContext(nc))

        persist = ctx.enter_context(tc.tile_pool(name="persist", bufs=1))
        dram = ctx.enter_context(tc.tile_pool(name="dram", bufs=1, space="DRAM"))
        # st 2x2 banks + yp0 + yp1 + qps 2 = 8 banks
        ps = ctx.enter_context(tc.tile_pool(name="ps", bufs=1, space="PSUM"))
        att = ctx.enter_context(tc.tile_pool(name="att", bufs=1))

        # ---- persistent SBUF tiles -----------------------------------
        wqk_sb = [persist.tile([P, n_cc, P], BF16, tag=f"wqk{f}",
                               name=f"wqk{f}") for f in range(n_f)]
        wv_sb = persist.tile([P, n_cc, CL], BF16, tag="wv", name="wv")
        wp_sb = persist.tile([P, n_cc, CL], BF16, tag="wp", name="wp")
        x_sb = [persist.tile([P, T], BF16, tag=f"x{c}", name=f"x{c}")
                for c in range(n_cc)]
        qk_sb = [persist.tile([P, T], BF16, tag=f"qk{f}", name=f"qk{f}")
                 for f in range(n_f)]
        v_sb = [persist.tile([P, HL, HS + 2], BF16, tag=f"v{t}",
                             name=f"v{t}") for t in range(n_kt)]
        bqk_sb = persist.tile([P, n_f], F32, tag="bqk", name="bqk_sb")
        bv_bc = persist.tile([P, CL], F32, tag="bv_bc", name="bv_bc")
        bp_bc = persist.tile([P, CL], F32, tag="bp_bc", name="bp_bc")
        ones_f = persist.tile([P, HL, 1], F32, tag="ones_f", name="ones_f")
        # tri[p, g] = 1 where g >= p else 0 (keep-at-or-above-diagonal)
        tri = persist.tile([P, P], BF16, tag="tri", name="tri")

        # ---- input DMAs: priority order, sync+scalar queues only ------
        # (only sync/gpsimd/scalar can initiate DMAs.) Every logical
        # queue sprays across all 16 DMA engines; what matters is
        # per-queue ORDER. First-matmul deps go first.
        # gpsimd carries NO bulk inputs: its first compute op after DMA
        # triggers forces a pool reconfig that waits for ALL its queued
        # transfers to drain — the preamble below (which gates all of
        # attention: tri mask, bias broadcasts) would stall ~25us.
        qs = [nc.sync, nc.scalar]
        nc.sync.dma_start(bqk_sb[:], bqk)
        bv_row = att.tile([1, CL], F32, tag="brow", bufs=2, name="bv_row")
        nc.sync.dma_start(bv_row[:], bv.rearrange("(o c) -> o c", o=1))
        bp_row = att.tile([1, CL], F32, tag="brow", bufs=2, name="bp_row")
        nc.scalar.dma_start(bp_row[:], bp.rearrange("(o c) -> o c", o=1))

        # gpsimd preamble right away (waits only on the two bias rows)
        nc.gpsimd.partition_broadcast(bv_bc[:], bv_row[:])
        nc.gpsimd.partition_broadcast(bp_bc[:], bp_row[:])
        nc.gpsimd.memset(ones_f[:], 1.0)
        nc.gpsimd.memset(tri[:], 1.0)
        nc.gpsimd.affine_select(
            out=tri[:], in_=tri[:], compare_op=mybir.AluOpType.is_ge,
            fill=0.0, base=0, channel_multiplier=-1, pattern=[[1, P]])

        # Early bulk (needed in the first ~35us) rides all 3 queues —
        # gpsimd's transfers here all complete before its first
        # normalize broadcast (~50us), so the pool-reconfig drain-wait
        # is a no-op. Late bulk (x q2/q3, wp) stays OFF gpsimd.
        q3_ = [nc.sync, nc.scalar, nc.gpsimd]
        hc = n_cc // 2
        nc.sync.dma_start(wqk_sb[0][:, 0:hc], wqk[0, :, 0:hc])
        nc.gpsimd.dma_start(wqk_sb[0][:, hc:], wqk[0, :, hc:])
        nc.scalar.dma_start(wqk_sb[n_jt][:, 0:hc], wqk[n_jt, :, 0:hc])
        nc.gpsimd.dma_start(wqk_sb[n_jt][:, hc:], wqk[n_jt, :, hc:])
        # x quarter 0 striped
        ri = 0
        for c in range(n_cc):
            q3_[ri % 3].dma_start(x_sb[c][:, 0:QT],
                                  xT[c * P:(c + 1) * P, 0:QT])
            ri += 1
        # wv quarters next (v0-3 pop inside tile (0,0); its matmuls
        # consume wv c-chunk by c-chunk as these land)
        nc.sync.dma_start(wv_sb[:, 0:2], wv_s[:, 0:2])
        nc.scalar.dma_start(wv_sb[:, 2:4], wv_s[:, 2:4])
        nc.gpsimd.dma_start(wv_sb[:, 4:6], wv_s[:, 4:6])
        nc.sync.dma_start(wv_sb[:, 6:8], wv_s[:, 6:8])
        # wqk f=1,5: round-0 fillers qk(f, 0) for pairs 1-2 pop early and
        # their matmuls head-of-line-block the PE queue until these land
        nc.scalar.dma_start(wqk_sb[1][:], wqk[1])
        nc.gpsimd.dma_start(wqk_sb[5][:], wqk[5])
        for f in (2, 6, 3, 7):
            q3_[ri % 3].dma_start(wqk_sb[f][:], wqk[f])
            ri += 1
        # x quarter 1 still on all three (needed ~30us)
        for c in range(n_cc):
            q3_[ri % 3].dma_start(x_sb[c][:, QT:2 * QT],
                                  xT[c * P:(c + 1) * P, QT:2 * QT])
            ri += 1
        # late bulk: sync+scalar only
        for q4 in range(2, 4):
            for c in range(n_cc):
                qs[ri % 2].dma_start(
                    x_sb[c][:, q4 * QT:(q4 + 1) * QT],
                    xT[c * P:(c + 1) * P, q4 * QT:(q4 + 1) * QT])
                ri += 1
            if q4 == 2:
                nc.scalar.dma_start(wp_sb[:, 0:hc], wp_s[:, 0:hc])
                nc.sync.dma_start(wp_sb[:, hc:], wp_s[:, hc:])

        # ---- AG segments ---------------------------------------------
        # per pair: one T-half segment for j=0/1, quarter segments for
        # j=2 and j=3 -> 12 AllGathers, staggered across rounds.
        segs = {}   # (pr, j) -> [ti, to, col_base, j_set]
        for pr in range(n_pair):
            ti = dram.tile([P, TH], BF16, tag=f"agi{pr}_h0", name=f"agi{pr}_h0")
            to = dram.tile([2, P, TH], BF16, tag=f"ago{pr}_h0",
                           name=f"ago{pr}_h0")
            for j in (0, 1):
                segs[(pr, j)] = [ti, to, j * QT, {0, 1}]
            for j in (2, 3):
                ti = dram.tile([P, QT], BF16, tag=f"agi{pr}_q{j}",
                               name=f"agi{pr}_q{j}")
                to = dram.tile([2, P, QT], BF16, tag=f"ago{pr}_q{j}",
                               name=f"ago{pr}_q{j}")
                segs[(pr, j)] = [ti, to, 0, {j}]

        # ---- compute atoms -------------------------------------------
        def v_atom(t):
            """V for t-chunk t: [128 t, CL] + bias, ones col per head."""
            pv = ps.tile([P, CL], F32, tag="qps", bufs=2, name="pv")
            for c in range(n_cc):
                nc.tensor.matmul(
                    pv[:], x_sb[c][:, t * KC:(t + 1) * KC], wv_sb[:, c, :],
                    start=(c == 0), stop=(c == n_cc - 1))
            nc.vector.tensor_copy(v_sb[t][:, :, HS:HS + 1], ones_f[:])
            nc.vector.tensor_add(
                v_sb[t][:, :, 0:HS],
                pv.rearrange("p (h e) -> p h e", e=HS),
                bv_bc.rearrange("p (h e) -> p h e", e=HS))

        def qk_atom(f, t):
            """q/k f-tile x one t-tile of 512: 8 matmuls + bias to SBUF."""
            pq = ps.tile([P, QT], F32, tag="qps", bufs=2, name="pq")
            for c in range(n_cc):
                nc.tensor.matmul(
                    pq[:], wqk_sb[f][:, c, :],
                    x_sb[c][:, t * QT:(t + 1) * QT],
                    start=(c == 0), stop=(c == n_cc - 1))
            nc.vector.tensor_scalar_add(
                qk_sb[f][:, t * QT:(t + 1) * QT], pq[:], bqk_sb[:, f:f + 1])

        # yr[(pr, j, gp)] = SBUF tile with replica gp's y^T block for
        # q-tile j of pair pr ([128 feat, 512 q], fetched post-AG).
        # Fetches ride the filler queue (gated a couple of positions
        # after their AG) so their AG-completion wait never head-of-line
        # blocks a latency-critical engine queue; sync hosts them (the
        # gpsimd queue must stay clear for normalize broadcasts and AG
        # triggers, scalar for the exp chain).
        yr = {}

        def yr_fetch(pr, j):
            _, to, col_base, _ = segs[(pr, j)]
            for gp in range(2):
                t_ = att.tile([P, QT], BF16, tag="yr", bufs=32, name="yr")
                nc.sync.dma_start(t_[:], to[gp, :, col_base:col_base + QT])
                yr[(pr, j, gp)] = t_

        def proj_atom(t):
            """Projection for t-chunk t: 8 matmuls PSUM-accum + bias add.

            Split into a pr 0-2 group and a pr 3 group: pair 3's AG is
            always the last to land, and one 8-matmul group would hold
            its PSUM bank (and stall the qps pool) for the whole wait.
            """
            j = t // KPQ
            col = (t % KPQ) * P
            oc = att.tile([P, CL], F32, tag="oacc", bufs=4, name="oacc")
            po = ps.tile([P, CL], F32, tag="qps", bufs=2, name="po")
            k = 0
            for pr in range(n_pair - 1):
                for gp in range(2):
                    nc.tensor.matmul(
                        po[:], yr[(pr, j, gp)][:, col:col + P],
                        wp_sb[:, gp * n_pair + pr, :],
                        start=(k == 0), stop=(k == 2 * (n_pair - 1) - 1))
                    k += 1
            nc.vector.tensor_add(oc[:], po[:], bp_bc[:])
            po3 = ps.tile([P, CL], F32, tag="qps", bufs=2, name="po3")
            for gp in range(2):
                nc.tensor.matmul(
                    po3[:], yr[(n_pair - 1, j, gp)][:, col:col + P],
                    wp_sb[:, gp * n_pair + n_pair - 1, :],
                    start=(gp == 0), stop=(gp == 1))
            nc.vector.tensor_add(oc[:], oc[:], po3[:])
            # keep outputs off sync mid-kernel (ti writes + fetches live
            # there); scalar only at the drain, when exp is done
            oq = nc.scalar if t >= 12 else (nc.gpsimd if t % 2 else nc.sync)
            oq.dma_start(out_ext[t * P:(t + 1) * P, :], oc[:])

        # ---- filler queue --------------------------------------------
        # (min_pos, thunk): position = j*4 + pr of the attention tile at
        # or after which the atom may be emitted.
        filler = []
        # v0-3 ride the filler queue (popped inside tile (0,0) after its
        # first score matmuls): emitted upfront, their late-arriving wv
        # DMA head-of-line-blocks the PE queue before attention can start
        for t in range(4):
            filler.append((0, lambda t=t: v_atom(t)))
        for f in (1, 5):
            filler.append((0, lambda f=f: qk_atom(f, 0)))
        for f in (2, 6):
            filler.append((1, lambda f=f: qk_atom(f, 0)))
        for f in (3, 7):
            filler.append((1, lambda f=f: qk_atom(f, 0)))
        # round r+1 deps staged across the later tiles of round r (the
        # x quarter r+1 DMAs land mid-round; popping these too early
        # head-of-line-blocks the PE queue on the DMA semaphore).
        # v t-chunks 8-15 are only consumed by chunk i>=8 of their round,
        # so they slide INTO rounds 2/3 as PE filler for the exp-bound
        # stretches there (all four must pop within the round's FIRST
        # tile, which itself consumes them at chunks i>=8 / 12).
        # v4-7 and the qk t=1 batch wait for x quarter 1 (~33us): gate
        # them at pos 3 so their pops don't block the PE queue earlier
        for k in range(4):
            filler.append((3, lambda t=4 + k: v_atom(t)))
        for k, f in enumerate((0, 4, 1, 5, 2, 6, 3, 7)):
            filler.append((3, lambda f=f: qk_atom(f, 1)))
        for k in range(4):
            filler.append((8, lambda t=8 + k: v_atom(t)))
        for k in range(4):
            filler.append((12, lambda t=12 + k: v_atom(t)))
        for r in range(1, 3):
            t = r + 1
            for k, f in enumerate((0, 4, 1, 5, 2, 6, 3, 7)):
                filler.append((4 * r + 2 + k // 4,
                               lambda f=f, t=t: qk_atom(f, t)))
        # yr fetches: gated well after their AG fires — AG *execution*
        # lags its trigger by up to ~25us (CC-stream serialization plus
        # inter-core skew: the collective starts only when BOTH cores of
        # the pair arrive). A fetch popped before its AG finished would
        # head-of-line-block sync, delaying later ti writes and
        # cascading into the q3 AG chain.
        for pr in range(n_pair):
            filler.append((10 + pr, lambda pr=pr: yr_fetch(pr, 1)))
            filler.append((10 + pr, lambda pr=pr: yr_fetch(pr, 0)))
        for pr, g in ((0, 13), (1, 14), (2, 14), (3, 15)):
            filler.append((g, lambda pr=pr: yr_fetch(pr, 2)))
        for pr, g in ((0, 15), (1, 15), (2, 16), (3, 16)):
            filler.append((g, lambda pr=pr: yr_fetch(pr, 3)))
        # projection: h0 t-chunks fill late round 3 (the exp-bound
        # stretch with the least native PE work); q2 starts inside the
        # last tile, the rest drains — pr 0-2 matmul groups overlap the
        # final AG, pr 3 groups follow its fetch.
        for t in range(8):
            filler.append((14 if t < 4 else 15, lambda t=t: proj_atom(t)))
        for t in range(8, 16):
            filler.append((15 if t < 10 else 16, lambda t=t: proj_atom(t)))

        def pop_filler(pos):
            for idx, (mp, thunk) in enumerate(filler):
                if mp <= pos:
                    filler.pop(idx)
                    thunk()
                    return True
            return False

        # ---- attention tile ------------------------------------------
        def att_tile(pr, j, pos):
            """Both heads of pair pr on q-tile j; 2-deep st -> exp -> av."""
            kT = qk_sb[n_pair + pr]
            qTt = qk_sb[pr]
            yps = {rr: ps.tile([P, QT], F32, tag=f"yp{rr}", bufs=1,
                               name=f"yp{rr}") for rr in range(2)}
            imax = KPQ * j + KPQ
            if pos > 0:
                pop_filler(pos)
                pop_filler(pos)
                pop_filler(pos)
            pend = []   # pipelined (i, off, pt) awaiting av

            def av(iv, offv, ptv):
                for rr in range(2):
                    nc.tensor.matmul(
                        yps[rr][0:HS + 1, offv:QT],
                        v_sb[iv][:, 2 * pr + rr, 0:HS + 1],
                        ptv[:, rr, offv:QT],
                        start=(iv == 0), stop=(iv == imax - 1))

            for i in range(imax):
                diag = (i // KPQ == j)
                # causally trim diagonal chunks to q >= i*KC
                off = KC * (i % KPQ) if diag else 0
                st = ps.tile([P, 2, QT], F32, tag="st", bufs=2, name="st")
                for rr in range(2):
                    ro = HS * rr
                    nc.tensor.matmul(
                        st[:, rr, off:QT],
                        kT[ro:ro + HS, i * KC:(i + 1) * KC],
                        qTt[ro:ro + HS, j * QT + off:(j + 1) * QT],
                        start=True, stop=True)
                pt = att.tile([P, 2, QT], BF16, tag="pt", bufs=4, name="pt")
                nc.scalar.activation(
                    pt[:, :, off:QT], st[:, :, off:QT],
                    mybir.ActivationFunctionType.Exp, scale=scale)
                if diag:
                    for rr in range(2):
                        # zero above the diagonal in the leading 128x128
                        # triangle, in place
                        nc.vector.tensor_mul(
                            pt[:, rr, off:off + KC],
                            pt[:, rr, off:off + KC], tri[:])
                pend.append((i, off, pt))
                if len(pend) > 2:
                    av(*pend.pop(0))
                if i % 2 == 1 or pos == 0:
                    pop_filler(pos)
            while pend:
                av(*pend.pop(0))

            # normalize both heads into one y2 tile (rr=1 write is
            # partition-shifted 0->64; validated on HW)
            y2 = att.tile([P, QT], BF16, tag="y2", bufs=3, name="y2")
            for rr in range(2):
                # keep this chain at exactly these 4 ops (see docstring)
                row = att.tile([1, QT], F32, tag="row", bufs=3, name="row")
                nc.vector.tensor_copy(row[:], yps[rr][HS:HS + 1, :])
                rec = att.tile([1, QT], F32, tag="rec", bufs=3, name="rec")
                nc.vector.reciprocal_approx_fast(rec[:], row[:])
                rb = att.tile([HS, QT], F32, tag="rb", bufs=3, name="rb")
                nc.gpsimd.partition_broadcast(rb[:], rec[:])
                nc.vector.tensor_mul(y2[rr * HS:(rr + 1) * HS, :],
                                     yps[rr][0:HS, :], rb[:])
            ti, to, col_base, j_set = segs[(pr, j)]
            nc.sync.dma_start(ti[:, col_base:col_base + QT], y2[:])
            if j == max(j_set):
                nc.gpsimd.collective_compute(
                    "AllGather", mybir.AluOpType.bypass,
                    replica_groups=PAIRS,
                    ins=[ti.opt()], outs=[to.opt()])

        # ---- schedule ------------------------------------------------
        # minimal upfront: what round 0 pair 0 needs; everything else
        # flows in through the filler queue
        qk_atom(0, 0)
        qk_atom(n_jt, 0)
        for t in range(4):
            v_atom(t)

        for j in range(n_jt):
            for pr in range(n_pair):
                att_tile(pr, j, j * n_pair + pr)
        while pop_filler(4 * n_jt):
            pass

    nc.compile()
    return nc


def shard_inputs(x, w_attn, b_attn, w_proj, b_proj):
    """Slice/transpose/shuffle full inputs into 8 per-core input maps."""
    Bq, T, C = x.shape
    CL = C // 2
    n_cc = C // P
    n_f = 2 * CL // P
    bf = ml_dtypes.bfloat16
    in_maps = []
    for i in range(N_CORES):
        b, g = i // 2, i % 2
        sl = slice(CL * g, CL * (g + 1))
        wq = w_attn[:, sl]
        wk = w_attn[:, C + CL * g:C + CL * (g + 1)]
        wvv = w_attn[:, 2 * C + CL * g:2 * C + CL * (g + 1)]
        wqk = np.concatenate([wq, wk], axis=1)          # [C, 2CL]
        # [C, 2CL] -> [f, p, c, m]: row r = c*128+p, col = f*128+m
        wqk_s = np.ascontiguousarray(
            wqk.reshape(n_cc, P, n_f, P).transpose(2, 1, 0, 3)).astype(bf)
        wv_shuf = np.ascontiguousarray(
            wvv.reshape(n_cc, P, CL).transpose(1, 0, 2)).astype(bf)
        wp_shuf = np.ascontiguousarray(
            w_proj[:, sl].reshape(n_cc, P, CL).transpose(1, 0, 2)).astype(bf)
        in_maps.append({
            "xT": np.ascontiguousarray(x[b].T).astype(bf),
            "wqk": wqk_s,
            "wv_s": wv_shuf,
            "wp_s": wp_shuf,
            "bqk": np.ascontiguousarray(
                np.concatenate([b_attn[sl],
                                b_attn[C + CL * g:C + CL * (g + 1)]])
                .reshape(n_f, P).T),
            "bv": np.ascontiguousarray(b_attn[2 * C + CL * g:2 * C + CL * (g + 1)]),
            "bp": np.ascontiguousarray(b_proj[sl]),
        })
    return in_maps


def gather_outputs(results, B, T, C):
    CL = C // 2
    out = np.empty((B, T, C), dtype=np.float32)
    for i in range(N_CORES):
        b, g = i // 2, i % 2
        out[b, :, CL * g:CL * (g + 1)] = results[i]["out"]
    return out


_NC_CACHE = {}


def get_nc(T, C):
    key = (T, C)
    if key not in _NC_CACHE:
        _NC_CACHE[key] = build_nc(T=T, C=C, HL=C // HS // 2)
    return _NC_CACHE[key]


def kernel(x, w_attn, b_attn, w_proj, b_proj):
    x = np.asarray(x, dtype=np.float32)
    w_attn = np.asarray(w_attn, dtype=np.float32)
    b_attn = np.asarray(b_attn, dtype=np.float32)
    w_proj = np.asarray(w_proj, dtype=np.float32)
    b_proj = np.asarray(b_proj, dtype=np.float32)

    Bq, T, C = x.shape
    nc = get_nc(T, C)

    in_maps = shard_inputs(x, w_attn, b_attn, w_proj, b_proj)
    trace = os.environ.get("KERNEL_TRACE", "0") == "1"
    res = bass_utils.run_bass_kernel_spmd(
        nc, in_maps, core_ids=list(range(N_CORES)), trace=trace)
    if trace and res.exec_time_ns is not None:
        print(f"HW exec time: {res.exec_time_ns} ns", flush=True)
        kernel.last_exec_time_ns = res.exec_time_ns
        kernel.last_results = res
    return gather_outputs(res.results, Bq, T, C)


# revision 30
# speedup vs baseline: 1.0211x; 1.0211x over previous
"""Causal self-attention (B=4, T=2048, C=1024, H=16) on 8 TRN2 NeuronCores.

Sharding: data-parallel on batch (4) x tensor-parallel on heads (2 groups of
8). Core i handles batch i//2 and head-group i%2.

Schedule: j-OUTER rounds — for each q-tile j (512 wide), all 4 head-pairs
attend in sequence. This staggers the pairwise AllGathers across the whole
kernel instead of stacking them at the tail (the prior pair-outer schedule
spent its last ~50us at half HAM clock waiting on the final pair's AG chain).

Per (pair, j) tile: scores^T [k, q] stripes, 2-deep pipelined st -> exp ->
av; causal diagonal blocks N-trimmed + 128x128 triangle masked by DVE mul
with a 0/1 mask. exp is safe without max subtraction (scores*hs^-0.5 are
O(+-10)). sum(exp) rides as a ones column in v (av out M=65). The two
heads' score matmuls land on PE row tiles (0,0)/(64,0) and run concurrent.

Normalize: per head the exact 4-op chain (stage row to base-0 SBUF, fast
reciprocal, gpsimd partition_broadcast, mul) — do not add ops; a 5-op chain
collapses the HW clock. rr=0 mul writes y2[0:64], rr=1 writes y2[64:128]
(partition-shifted DVE write, HW-validated), then ONE dma lands y2 in the
AG input segment.

Collectives: 12 pairwise AllGathers — per pair one T-half segment for
j=0/1 and quarter segments for j=2, j=3. After each AG, yr tiles ([128,512]
per (pair, j, group), 1KB DMA lines) are fetched to SBUF; projection for
t-chunk t accumulates 8 matmuls (4 pairs x 2 groups) into one PSUM bank,
one DVE add folds b_proj, out DMA rotates across queues. Output lands
spread over the second half of the kernel instead of all at the end.

Startup: input DMAs priority-ordered across 5 trigger queues (sync, gpsimd,
scalar, vector, tensor): first-matmul deps (bqk, wqk[0]/wqk[4] halves,
x quarter 0) first, then wv, then the rest in consumption order.

dtypes: matmul operands bf16, accumulation fp32 in PSUM, softmax
normalization fp32. (fp8 anywhere fails the 2e-2 gate: ~6% y error.)

HW gotchas (CoreSim passes these; only real HW fails):
  - a single 65-partition DVE copy from PSUM silently corrupts data
  - gpsimd custom-DVE reciprocal misreads PSUM and misreads inputs at a
    nonzero base partition (stage rows into base-0 SBUF first)
  - junk "heater" matmuls and 16x fine-grained AllGathers both regress
"""

import os
import sys
from contextlib import ExitStack

import numpy as np
import ml_dtypes

if "/opt/trn_rl_repo" not in sys.path:
    sys.path.insert(0, "/opt/trn_rl_repo")

import concourse.bass as bass
import concourse.mybir as mybir
import concourse.tile as tile
from concourse import bacc
from concourse import bass_utils

F32 = mybir.dt.float32
BF16 = mybir.dt.bfloat16
P = 128          # SBUF partitions
QT = 512         # q tile (matmul free dim)
KC = 128         # k chunk (psum partition dim)
HS = 64          # head size
KPQ = QT // KC   # k chunks per q tile

N_CORES = 8
PAIRS = [[0, 1], [2, 3], [4, 5], [6, 7]]

B_FULL, T_FULL, C_FULL, H_FULL = 4, 2048, 1024, 16


def build_nc(T=T_FULL, C=C_FULL, HL=H_FULL // 2):
    """Build the SPMD graph for one core (all 8 cores run the same graph).

    Per-core input tensors:
      xT    [C, T] bf16       x[b] transposed
      wqk   [2CL/P, P, C/P, P] bf16  w_attn q|k cols, host-shuffled [f,p,c,m]
      wv_s  [P, C/P, CL] bf16 w_attn v cols, host-shuffled [p,c,m]
      wp_s  [P, C/P, CL] bf16 w_proj (all 1024 rows as 2 groups x 4 pairs,
                              this core's 512 out cols), host-shuffled
      bqk   [P, 2*CL/P] f32, bv [CL] f32, bp [CL] f32
    Output: out [T, CL] f32.
    """
    CL = HL * HS                 # local width (q, k, v, out-cols each)
    n_cc = C // P                # x feature chunks (8)
    n_f = 2 * CL // P            # q|k f-tiles (4 q then 4 k)
    n_jt = T // QT               # q tiles / rounds (4)
    n_kt = T // KC               # k chunks / v t-chunks (16)
    n_pair = HL // 2             # head pairs (4)
    TH = T // 2
    scale = HS ** -0.5

    nc = bacc.Bacc("TRN2", target_bir_lowering=False, debug=False,
                   num_devices=N_CORES)

    xT = nc.dram_tensor("xT", [C, T], BF16, kind="ExternalInput").ap()
    wqk = nc.dram_tensor("wqk", [n_f, P, n_cc, P], BF16,
                         kind="ExternalInput").ap()
    wv_s = nc.dram_tensor("wv_s", [P, n_cc, CL], BF16,
                          kind="ExternalInput").ap()
    wp_s = nc.dram_tensor("wp_s", [P, n_cc, CL], BF16,
                          kind="ExternalInput").ap()
    bqk = nc.dram_tensor("bqk", [P, 2 * CL // P], F32,
                         kind="ExternalInput").ap()
    bv = nc.dram_tensor("bv", [CL], F32, kind="ExternalInput").ap()
    bp = nc.dram_tensor("bp", [CL], F32, kind="ExternalInput").ap()
    out_ext = nc.dram_tensor("out", [T, CL], F32, kind="ExternalOutput").ap()

    with ExitStack() as ctx:
        tc = ctx.enter_context(tile.TileContext(nc))

        persist = ctx.enter_context(tc.tile_pool(name="persist", bufs=1))
        dram = ctx.enter_context(tc.tile_pool(name="dram", bufs=1, space="DRAM"))
        # st 2x2 banks + yp0 + yp1 + qps 2 = 8 banks
        ps = ctx.enter_context(tc.tile_pool(name="ps", bufs=1, space="PSUM"))
        att = ctx.enter_context(tc.tile_pool(name="att", bufs=1))

        # ---- persistent SBUF tiles -----------------------------------
        wqk_sb = [persist.tile([P, n_cc, P], BF16, tag=f"wqk{f}",
                               name=f"wqk{f}") for f in range(n_f)]
        wv_sb = persist.tile([P, n_cc, CL], BF16, tag="wv", name="wv")
        wp_sb = persist.tile([P, n_cc, CL], BF16, tag="wp", name="wp")
        x_sb = [persist.tile([P, T], BF16, tag=f"x{c}", name=f"x{c}")
                for c in range(n_cc)]
        qk_sb = [persist.tile([P, T], BF16, tag=f"qk{f}", name=f"qk{f}")
                 for f in range(n_f)]
        v_sb = [persist.tile([P, HL, HS + 2], BF16, tag=f"v{t}",
                             name=f"v{t}") for t in range(n_kt)]
        bqk_sb = persist.tile([P, n_f], F32, tag="bqk", name="bqk_sb")
        bv_bc = persist.tile([P, CL], F32, tag="bv_bc", name="bv_bc")
        bp_bc = persist.tile([P, CL], F32, tag="bp_bc", name="bp_bc")
        ones_f = persist.tile([P, HL, 1], F32, tag="ones_f", name="ones_f")
        # tri[p, g] = 1 where g >= p else 0 (keep-at-or-above-diagonal)
        tri = persist.tile([P, P], BF16, tag="tri", name="tri")

        # ---- input DMAs: priority order, sync+scalar queues only ------
        # (only sync/gpsimd/scalar can initiate DMAs.) Every logical
        # queue sprays across all 16 DMA engines; what matters is
        # per-queue ORDER. First-matmul deps go first.
        # gpsimd carries NO bulk inputs: its first compute op after DMA
        # triggers forces a pool reconfig that waits for ALL its queued
        # transfers to drain — the preamble below (which gates all of
        # attention: tri mask, bias broadcasts) would stall ~25us.
        qs = [nc.sync, nc.scalar]
        nc.sync.dma_start(bqk_sb[:], bqk)
        bv_row = att.tile([1, CL], F32, tag="brow", bufs=2, name="bv_row")
        nc.sync.dma_start(bv_row[:], bv.rearrange("(o c) -> o c", o=1))
        bp_row = att.tile([1, CL], F32, tag="brow", bufs=2, name="bp_row")
        nc.scalar.dma_start(bp_row[:], bp.rearrange("(o c) -> o c", o=1))

        # gpsimd preamble right away (waits only on the two bias rows)
        nc.gpsimd.partition_broadcast(bv_bc[:], bv_row[:])
        nc.gpsimd.partition_broadcast(bp_bc[:], bp_row[:])
        nc.gpsimd.memset(ones_f[:], 1.0)
        nc.gpsimd.memset(tri[:], 1.0)
        nc.gpsimd.affine_select(
            out=tri[:], in_=tri[:], compare_op=mybir.AluOpType.is_ge,
            fill=0.0, base=0, channel_multiplier=-1, pattern=[[1, P]])

        # Early bulk (needed in the first ~35us) rides all 3 queues —
        # gpsimd's transfers here all complete before its first
        # normalize broadcast (~50us), so the pool-reconfig drain-wait
        # is a no-op. Late bulk (x q2/q3, wp) stays OFF gpsimd.
        q3_ = [nc.sync, nc.scalar, nc.gpsimd]
        hc = n_cc // 2
        nc.sync.dma_start(wqk_sb[0][:, 0:hc], wqk[0, :, 0:hc])
        nc.gpsimd.dma_start(wqk_sb[0][:, hc:], wqk[0, :, hc:])
        nc.scalar.dma_start(wqk_sb[n_jt][:, 0:hc], wqk[n_jt, :, 0:hc])
        nc.gpsimd.dma_start(wqk_sb[n_jt][:, hc:], wqk[n_jt, :, hc:])
        # x quarter 0 striped
        ri = 0
        for c in range(n_cc):
            q3_[ri % 3].dma_start(x_sb[c][:, 0:QT],
                                  xT[c * P:(c + 1) * P, 0:QT])
            ri += 1
        # wv quarters next (v0-3 pop inside tile (0,0); its matmuls
        # consume wv c-chunk by c-chunk as these land)
        nc.sync.dma_start(wv_sb[:, 0:2], wv_s[:, 0:2])
        nc.scalar.dma_start(wv_sb[:, 2:4], wv_s[:, 2:4])
        nc.gpsimd.dma_start(wv_sb[:, 4:6], wv_s[:, 4:6])
        nc.sync.dma_start(wv_sb[:, 6:8], wv_s[:, 6:8])
        # wqk f=1,5: round-0 fillers qk(f, 0) for pairs 1-2 pop early and
        # their matmuls head-of-line-block the PE queue until these land
        nc.scalar.dma_start(wqk_sb[1][:], wqk[1])
        nc.gpsimd.dma_start(wqk_sb[5][:], wqk[5])
        for f in (2, 6, 3, 7):
            q3_[ri % 3].dma_start(wqk_sb[f][:], wqk[f])
            ri += 1
        # x quarter 1 still on all three (needed ~30us)
        for c in range(n_cc):
            q3_[ri % 3].dma_start(x_sb[c][:, QT:2 * QT],
                                  xT[c * P:(c + 1) * P, QT:2 * QT])
            ri += 1
        # late bulk: sync+scalar only
        for q4 in range(2, 4):
            for c in range(n_cc):
                qs[ri % 2].dma_start(
                    x_sb[c][:, q4 * QT:(q4 + 1) * QT],
                    xT[c * P:(c + 1) * P, q4 * QT:(q4 + 1) * QT])
                ri += 1
            if q4 == 2:
                nc.scalar.dma_start(wp_sb[:, 0:hc], wp_s[:, 0:hc])
                nc.sync.dma_start(wp_sb[:, hc:], wp_s[:, hc:])

        # ---- AG segments ---------------------------------------------
        # per pair: one T-half segment for j=0/1, quarter segments for
        # j=2 and j=3 -> 12 AllGathers, staggered across rounds.
        segs = {}   # (pr, j) -> [ti, to, col_base, j_set]
        for pr in range(n_pair):
            ti = dram.tile([P, TH], BF16, tag=f"agi{pr}_h0", name=f"agi{pr}_h0")
            to = dram.tile([2, P, TH], BF16, tag=f"ago{pr}_h0",
                           name=f"ago{pr}_h0")
            for j in (0, 1):
                segs[(pr, j)] = [ti, to, j * QT, {0, 1}]
            for j in (2, 3):
                ti = dram.tile([P, QT], BF16, tag=f"agi{pr}_q{j}",
                               name=f"agi{pr}_q{j}")
                to = dram.tile([2, P, QT], BF16, tag=f"ago{pr}_q{j}",
                               name=f"ago{pr}_q{j}")
                segs[(pr, j)] = [ti, to, 0, {j}]

        # ---- compute atoms -------------------------------------------
        def v_atom(t):
            """V for t-chunk t: [128 t, CL] + bias, ones col per head."""
            pv = ps.tile([P, CL], F32, tag="qps", bufs=2, name="pv")
            for c in range(n_cc):
                nc.tensor.matmul(
                    pv[:], x_sb[c][:, t * KC:(t + 1) * KC], wv_sb[:, c, :],
                    start=(c == 0), stop=(c == n_cc - 1))
            nc.vector.tensor_copy(v_sb[t][:, :, HS:HS + 1], ones_f[:])
            nc.vector.tensor_add(
                v_sb[t][:, :, 0:HS],
                pv.rearrange("p (h e) -> p h e", e=HS),
                bv_bc.rearrange("p (h e) -> p h e", e=HS))

        def qk_atom(f, t):
            """q/k f-tile x one t-tile of 512: 8 matmuls + bias to SBUF."""
            pq = ps.tile([P, QT], F32, tag="qps", bufs=2, name="pq")
            for c in range(n_cc):
                nc.tensor.matmul(
                    pq[:], wqk_sb[f][:, c, :],
                    x_sb[c][:, t * QT:(t + 1) * QT],
                    start=(c == 0), stop=(c == n_cc - 1))
            nc.vector.tensor_scalar_add(
                qk_sb[f][:, t * QT:(t + 1) * QT], pq[:], bqk_sb[:, f:f + 1])

        # yr[(pr, j, gp)] = SBUF tile with replica gp's y^T block for
        # q-tile j of pair pr ([128 feat, 512 q], fetched post-AG).
        # Fetches ride the filler queue (gated a couple of positions
        # after their AG) so their AG-completion wait never head-of-line
        # blocks a latency-critical engine queue; sync hosts them (the
        # gpsimd queue must stay clear for normalize broadcasts and AG
        # triggers, scalar for the exp chain).
        yr = {}

        def yr_fetch(pr, j):
            _, to, col_base, _ = segs[(pr, j)]
            for gp in range(2):
                t_ = att.tile([P, QT], BF16, tag="yr", bufs=32, name="yr")
                nc.sync.dma_start(t_[:], to[gp, :, col_base:col_base + QT])
                yr[(pr, j, gp)] = t_

        def proj_atom(t):
            """Projection for t-chunk t: 8 matmuls PSUM-accum + bias add.

            Split into a pr 0-2 group and a pr 3 group: pair 3's AG is
            always the last to land, and one 8-matmul group would hold
            its PSUM bank (and stall the qps pool) for the whole wait.
            """
            j = t // KPQ
            col = (t % KPQ) * P
            oc = att.tile([P, CL], F32, tag="oacc", bufs=4, name="oacc")
            po = ps.tile([P, CL], F32, tag="qps", bufs=2, name="po")
            k = 0
            for pr in range(n_pair - 1):
                for gp in range(2):
                    nc.tensor.matmul(
                        po[:], yr[(pr, j, gp)][:, col:col + P],
                        wp_sb[:, gp * n_pair + pr, :],
                        start=(k == 0), stop=(k == 2 * (n_pair - 1) - 1))
                    k += 1
            nc.vector.tensor_add(oc[:], po[:], bp_bc[:])
            po3 = ps.tile([P, CL], F32, tag="qps", bufs=2, name="po3")
            for gp in range(2):
                nc.tensor.matmul(
                    po3[:], yr[(n_pair - 1, j, gp)][:, col:col + P],
                    wp_sb[:, gp * n_pair + n_pair - 1, :],
                    start=(gp == 0), stop=(gp == 1))
            nc.vector.tensor_add(oc[:], oc[:], po3[:])
            # keep outputs off sync mid-kernel (ti writes + fetches live
            # there); scalar only at the drain, when exp is done
            oq = nc.scalar if t >= 12 else (nc.gpsimd if t % 2 else nc.sync)
            oq.dma_start(out_ext[t * P:(t + 1) * P, :], oc[:])

        # ---- filler queue --------------------------------------------
        # (min_pos, thunk): position = j*4 + pr of the attention tile at
        # or after which the atom may be emitted.
        filler = []
        # v0-3 ride the filler queue (popped inside tile (0,0) after its
        # first score matmuls): emitted upfront, their late-arriving wv
        # DMA head-of-line-blocks the PE queue before attention can start
        for t in range(4):
            filler.append((0, lambda t=t: v_atom(t)))
        for f in (1, 5):
            filler.append((0, lambda f=f: qk_atom(f, 0)))
        for f in (2, 6):
            filler.append((1, lambda f=f: qk_atom(f, 0)))
        for f in (3, 7):
            filler.append((1, lambda f=f: qk_atom(f, 0)))
        # round r+1 deps staged across the later tiles of round r (the
        # x quarter r+1 DMAs land mid-round; popping these too early
        # head-of-line-blocks the PE queue on the DMA semaphore).
        # v t-chunks 8-15 are only consumed by chunk i>=8 of their round,
        # so they slide INTO rounds 2/3 as PE filler for the exp-bound
        # stretches there (all four must pop within the round's FIRST
        # tile, which itself consumes them at chunks i>=8 / 12).
        # v4-7 and the qk t=1 batch wait for x quarter 1 (~33us): gate
        # them at pos 3 so their pops don't block the PE queue earlier
        for k in range(4):
            filler.append((3, lambda t=4 + k: v_atom(t)))
        for k, f in enumerate((0, 4, 1, 5, 2, 6, 3, 7)):
            filler.append((2 + k // 4, lambda f=f: qk_atom(f, 1)))
        for k in range(4):
            filler.append((8, lambda t=8 + k: v_atom(t)))
        for k in range(4):
            filler.append((12, lambda t=12 + k: v_atom(t)))
        for r in range(1, 3):
            t = r + 1
            for k, f in enumerate((0, 4, 1, 5, 2, 6, 3, 7)):
                filler.append((4 * r + 2 + k // 4,
                               lambda f=f, t=t: qk_atom(f, t)))
        # yr fetches: gated well after their AG fires — AG *execution*
        # lags its trigger by up to ~25us (CC-stream serialization plus
        # inter-core skew: the collective starts only when BOTH cores of
        # the pair arrive). A fetch popped before its AG finished would
        # head-of-line-block sync, delaying later ti writes and
        # cascading into the q3 AG chain.
        for pr in range(n_pair):
            filler.append((10 + pr, lambda pr=pr: yr_fetch(pr, 1)))
            filler.append((10 + pr, lambda pr=pr: yr_fetch(pr, 0)))
        for pr, g in ((0, 13), (1, 14), (2, 14), (3, 15)):
            filler.append((g, lambda pr=pr: yr_fetch(pr, 2)))
        for pr, g in ((0, 15), (1, 15), (2, 16), (3, 16)):
            filler.append((g, lambda pr=pr: yr_fetch(pr, 3)))
        # projection: h0 t-chunks fill late round 3 (the exp-bound
        # stretch with the least native PE work); q2 starts inside the
        # last tile, the rest drains — pr 0-2 matmul groups overlap the
        # final AG, pr 3 groups follow its fetch.
        for t in range(8):
            filler.append((14 if t < 4 else 15, lambda t=t: proj_atom(t)))
        for t in range(8, 16):
            filler.append((15 if t < 10 else 16, lambda t=t: proj_atom(t)))

        def pop_filler(pos):
            for idx, (mp, thunk) in enumerate(filler):
                if mp <= pos:
                    filler.pop(idx)
                    thunk()
                    return True
            return False

        # ---- attention tile ------------------------------------------
        def att_tile(pr, j, pos):
            """Both heads of pair pr on q-tile j; 2-deep st -> exp -> av."""
            kT = qk_sb[n_pair + pr]
            qTt = qk_sb[pr]
            yps = {rr: ps.tile([P, QT], F32, tag=f"yp{rr}", bufs=1,
                               name=f"yp{rr}") for rr in range(2)}
            imax = KPQ * j + KPQ
            if pos > 0:
                pop_filler(pos)
                pop_filler(pos)
                pop_filler(pos)
            pend = []   # pipelined (i, off, pt) awaiting av

            def av(iv, offv, ptv):
                for rr in range(2):
                    nc.tensor.matmul(
                        yps[rr][0:HS + 1, offv:QT],
                        v_sb[iv][:, 2 * pr + rr, 0:HS + 1],
                        ptv[:, rr, offv:QT],
                        start=(iv == 0), stop=(iv == imax - 1))

            for i in range(imax):
                diag = (i // KPQ == j)
                # causally trim diagonal chunks to q >= i*KC
                off = KC * (i % KPQ) if diag else 0
                st = ps.tile([P, 2, QT], F32, tag="st", bufs=2, name="st")
                for rr in range(2):
                    ro = HS * rr
                    nc.tensor.matmul(
                        st[:, rr, off:QT],
                        kT[ro:ro + HS, i * KC:(i + 1) * KC],
                        qTt[ro:ro + HS, j * QT + off:(j + 1) * QT],
                        start=True, stop=True)
                pt = att.tile([P, 2, QT], BF16, tag="pt", bufs=4, name="pt")
                nc.scalar.activation(
                    pt[:, :, off:QT], st[:, :, off:QT],
                    mybir.ActivationFunctionType.Exp, scale=scale)
                if diag:
                    for rr in range(2):
                        # zero above the diagonal in the leading 128x128
                        # triangle, in place
                        nc.vector.tensor_mul(
                            pt[:, rr, off:off + KC],
                            pt[:, rr, off:off + KC], tri[:])
                pend.append((i, off, pt))
                if len(pend) > 2:
                    av(*pend.pop(0))
                if i % 2 == 1 or pos == 0:
                    pop_filler(pos)
            while pend:
                av(*pend.pop(0))

            # normalize both heads into one y2 tile (rr=1 write is
            # partition-shifted 0->64; validated on HW)
            y2 = att.tile([P, QT], BF16, tag="y2", bufs=3, name="y2")
            for rr in range(2):
                # keep this chain at exactly these 4 ops (see docstring)
                row = att.tile([1, QT], F32, tag="row", bufs=3, name="row")
                nc.vector.tensor_copy(row[:], yps[rr][HS:HS + 1, :])
                rec = att.tile([1, QT], F32, tag="rec", bufs=3, name="rec")
                nc.vector.reciprocal_approx_fast(rec[:], row[:])
                rb = att.tile([HS, QT], F32, tag="rb", bufs=3, name="rb")
                nc.gpsimd.partition_broadcast(rb[:], rec[:])
                nc.vector.tensor_mul(y2[rr * HS:(rr + 1) * HS, :],
                                     yps[rr][0:HS, :], rb[:])
            ti, to, col_base, j_set = segs[(pr, j)]
            nc.sync.dma_start(ti[:, col_base:col_base + QT], y2[:])
            if j == max(j_set):
                nc.gpsimd.collective_compute(
                    "AllGather", mybir.AluOpType.bypass,
                    replica_groups=PAIRS,
                    ins=[ti.opt()], outs=[to.opt()])

        # ---- schedule ------------------------------------------------
        # minimal upfront: what round 0 pair 0 needs; everything else
        # flows in through the filler queue
        qk_atom(0, 0)
        qk_atom(n_jt, 0)
        for t in range(4):
            v_atom(t)

        for j in range(n_jt):
            for pr in range(n_pair):
                att_tile(pr, j, j * n_pair + pr)
        while pop_filler(4 * n_jt):
            pass

    nc.compile()
    return nc


def shard_inputs(x, w_attn, b_attn, w_proj, b_proj):
    """Slice/transpose/shuffle full inputs into 8 per-core input maps."""
    Bq, T, C = x.shape
    CL = C // 2
    n_cc = C // P
    n_f = 2 * CL // P
    bf = ml_dtypes.bfloat16
    in_maps = []
    for i in range(N_CORES):
        b, g = i // 2, i % 2
        sl = slice(CL * g, CL * (g + 1))
        wq = w_attn[:, sl]
        wk = w_attn[:, C + CL * g:C + CL * (g + 1)]
        wvv = w_attn[:, 2 * C + CL * g:2 * C + CL * (g + 1)]
        wqk = np.concatenate([wq, wk], axis=1)          # [C, 2CL]
        # [C, 2CL] -> [f, p, c, m]: row r = c*128+p, col = f*128+m
        wqk_s = np.ascontiguousarray(
            wqk.reshape(n_cc, P, n_f, P).transpose(2, 1, 0, 3)).astype(bf)
        wv_shuf = np.ascontiguousarray(
            wvv.reshape(n_cc, P, CL).transpose(1, 0, 2)).astype(bf)
        wp_shuf = np.ascontiguousarray(
            w_proj[:, sl].reshape(n_cc, P, CL).transpose(1, 0, 2)).astype(bf)
        in_maps.append({
            "xT": np.ascontiguousarray(x[b].T).astype(bf),
            "wqk": wqk_s,
            "wv_s": wv_shuf,
            "wp_s": wp_shuf,
            "bqk": np.ascontiguousarray(
                np.concatenate([b_attn[sl],
                                b_attn[C + CL * g:C + CL * (g + 1)]])
                .reshape(n_f, P).T),
            "bv": np.ascontiguousarray(b_attn[2 * C + CL * g:2 * C + CL * (g + 1)]),
            "bp": np.ascontiguousarray(b_proj[sl]),
        })
    return in_maps


def gather_outputs(results, B, T, C):
    CL = C // 2
    out = np.empty((B, T, C), dtype=np.float32)
    for i in range(N_CORES):
        b, g = i // 2, i % 2
        out[b, :, CL * g:CL * (g + 1)] = results[i]["out"]
    return out


_NC_CACHE = {}


def get_nc(T, C):
    key = (T, C)
    if key not in _NC_CACHE:
        _NC_CACHE[key] = build_nc(T=T, C=C, HL=C // HS // 2)
    return _NC_CACHE[key]


def kernel(x, w_attn, b_attn, w_proj, b_proj):
    x = np.asarray(x, dtype=np.float32)
    w_attn = np.asarray(w_attn, dtype=np.float32)
    b_attn = np.asarray(b_attn, dtype=np.float32)
    w_proj = np.asarray(w_proj, dtype=np.float32)
    b_proj = np.asarray(b_proj, dtype=np.float32)

    Bq, T, C = x.shape
    nc = get_nc(T, C)

    in_maps = shard_inputs(x, w_attn, b_attn, w_proj, b_proj)
    trace = os.environ.get("KERNEL_TRACE", "0") == "1"
    res = bass_utils.run_bass_kernel_spmd(
        nc, in_maps, core_ids=list(range(N_CORES)), trace=trace)
    if trace and res.exec_time_ns is not None:
        print(f"HW exec time: {res.exec_time_ns} ns", flush=True)
        kernel.last_exec_time_ns = res.exec_time_ns
        kernel.last_results = res
    return gather_outputs(res.results, Bq, T, C)


# revision 32
# speedup vs baseline: 1.0227x; 1.0015x over previous
"""Causal self-attention (B=4, T=2048, C=1024, H=16) on 8 TRN2 NeuronCores.

Sharding: data-parallel on batch (4) x tensor-parallel on heads (2 groups of
8). Core i handles batch i//2 and head-group i%2.

Schedule: j-OUTER rounds — for each q-tile j (512 wide), all 4 head-pairs
attend in sequence. This staggers the pairwise AllGathers across the whole
kernel instead of stacking them at the tail (the prior pair-outer schedule
spent its last ~50us at half HAM clock waiting on the final pair's AG chain).

Per (pair, j) tile: scores^T [k, q] stripes, 2-deep pipelined st -> exp ->
av; causal diagonal blocks N-trimmed + 128x128 triangle masked by DVE mul
with a 0/1 mask. exp is safe without max subtraction (scores*hs^-0.5 are
O(+-10)). sum(exp) rides as a ones column in v (av out M=65). The two
heads' score matmuls land on PE row tiles (0,0)/(64,0) and run concurrent.

Normalize: per head the exact 4-op chain (stage row to base-0 SBUF, fast
reciprocal, gpsimd partition_broadcast, mul) — do not add ops; a 5-op chain
collapses the HW clock. rr=0 mul writes y2[0:64], rr=1 writes y2[64:128]
(partition-shifted DVE write, HW-validated), then ONE dma lands y2 in the
AG input segment.

Collectives: 12 pairwise AllGathers — per pair one T-half segment for
j=0/1 and quarter segments for j=2, j=3. After each AG, yr tiles ([128,512]
per (pair, j, group), 1KB DMA lines) are fetched to SBUF; projection for
t-chunk t accumulates 8 matmuls (4 pairs x 2 groups) into one PSUM bank,
one DVE add folds b_proj, out DMA rotates across queues. Output lands
spread over the second half of the kernel instead of all at the end.

Startup: input DMAs priority-ordered across 5 trigger queues (sync, gpsimd,
scalar, vector, tensor): first-matmul deps (bqk, wqk[0]/wqk[4] halves,
x quarter 0) first, then wv, then the rest in consumption order.

dtypes: matmul operands bf16, accumulation fp32 in PSUM, softmax
normalization fp32. (fp8 anywhere fails the 2e-2 gate: ~6% y error.)

HW gotchas (CoreSim passes these; only real HW fails):
  - a single 65-partition DVE copy from PSUM silently corrupts data
  - gpsimd custom-DVE reciprocal misreads PSUM and misreads inputs at a
    nonzero base partition (stage rows into base-0 SBUF first)
  - junk "heater" matmuls and 16x fine-grained AllGathers both regress
"""

import os
import sys
from contextlib import ExitStack

import numpy as np
import ml_dtypes

if "/opt/trn_rl_repo" not in sys.path:
    sys.path.insert(0, "/opt/trn_rl_repo")

import concourse.bass as bass
import concourse.mybir as mybir
import concourse.tile as tile
from concourse import bacc
from concourse import bass_utils

F32 = mybir.dt.float32
BF16 = mybir.dt.bfloat16
P = 128          # SBUF partitions
QT = 512         # q tile (matmul free dim)
KC = 128         # k chunk (psum partition dim)
HS = 64          # head size
KPQ = QT // KC   # k chunks per q tile

N_CORES = 8
PAIRS = [[0, 1], [2, 3], [4, 5], [6, 7]]

B_FULL, T_FULL, C_FULL, H_FULL = 4, 2048, 1024, 16


def build_nc(T=T_FULL, C=C_FULL, HL=H_FULL // 2):
    """Build the SPMD graph for one core (all 8 cores run the same graph).

    Per-core input tensors:
      xT    [C, T] bf16       x[b] transposed
      wqk   [2CL/P, P, C/P, P] bf16  w_attn q|k cols, host-shuffled [f,p,c,m]
      wv_s  [P, C/P, CL] bf16 w_attn v cols, host-shuffled [p,c,m]
      wp_s  [P, C/P, CL] bf16 w_proj (all 1024 rows as 2 groups x 4 pairs,
                              this core's 512 out cols), host-shuffled
      bqk   [P, 2*CL/P] f32, bv [CL] f32, bp [CL] f32
    Output: out [T, CL] f32.
    """
    CL = HL * HS                 # local width (q, k, v, out-cols each)
    n_cc = C // P                # x feature chunks (8)
    n_f = 2 * CL // P            # q|k f-tiles (4 q then 4 k)
    n_jt = T // QT               # q tiles / rounds (4)
    n_kt = T // KC               # k chunks / v t-chunks (16)
    n_pair = HL // 2             # head pairs (4)
    TH = T // 2
    scale = HS ** -0.5

    nc = bacc.Bacc("TRN2", target_bir_lowering=False, debug=False,
                   num_devices=N_CORES)

    xT = nc.dram_tensor("xT", [C, T], BF16, kind="ExternalInput").ap()
    wqk = nc.dram_tensor("wqk", [n_f, P, n_cc, P], BF16,
                         kind="ExternalInput").ap()
    wv_s = nc.dram_tensor("wv_s", [P, n_cc, CL], BF16,
                          kind="ExternalInput").ap()
    wp_s = nc.dram_tensor("wp_s", [P, n_cc, CL], BF16,
                          kind="ExternalInput").ap()
    bqk = nc.dram_tensor("bqk", [P, 2 * CL // P], F32,
                         kind="ExternalInput").ap()
    bv = nc.dram_tensor("bv", [CL], F32, kind="ExternalInput").ap()
    bp = nc.dram_tensor("bp", [CL], F32, kind="ExternalInput").ap()
    out_ext = nc.dram_tensor("out", [T, CL], F32, kind="ExternalOutput").ap()

    with ExitStack() as ctx:
        tc = ctx.enter_context(tile.TileContext(nc))

        persist = ctx.enter_context(tc.tile_pool(name="persist", bufs=1))
        dram = ctx.enter_context(tc.tile_pool(name="dram", bufs=1, space="DRAM"))
        # st 2x2 banks + yp0 + yp1 + qps 2 = 8 banks
        ps = ctx.enter_context(tc.tile_pool(name="ps", bufs=1, space="PSUM"))
        att = ctx.enter_context(tc.tile_pool(name="att", bufs=1))

        # ---- persistent SBUF tiles -----------------------------------
        wqk_sb = [persist.tile([P, n_cc, P], BF16, tag=f"wqk{f}",
                               name=f"wqk{f}") for f in range(n_f)]
        wv_sb = persist.tile([P, n_cc, CL], BF16, tag="wv", name="wv")
        wp_sb = persist.tile([P, n_cc, CL], BF16, tag="wp", name="wp")
        x_sb = [persist.tile([P, T], BF16, tag=f"x{c}", name=f"x{c}")
                for c in range(n_cc)]
        qk_sb = [persist.tile([P, T], BF16, tag=f"qk{f}", name=f"qk{f}")
                 for f in range(n_f)]
        v_sb = [persist.tile([P, HL, HS + 2], BF16, tag=f"v{t}",
                             name=f"v{t}") for t in range(n_kt)]
        bqk_sb = persist.tile([P, n_f], F32, tag="bqk", name="bqk_sb")
        bv_bc = persist.tile([P, CL], F32, tag="bv_bc", name="bv_bc")
        bp_bc = persist.tile([P, CL], F32, tag="bp_bc", name="bp_bc")
        ones_f = persist.tile([P, HL, 1], F32, tag="ones_f", name="ones_f")
        # tri[p, g] = 1 where g >= p else 0 (keep-at-or-above-diagonal)
        tri = persist.tile([P, P], BF16, tag="tri", name="tri")

        # ---- input DMAs: priority order, sync+scalar queues only ------
        # (only sync/gpsimd/scalar can initiate DMAs.) Every logical
        # queue sprays across all 16 DMA engines; what matters is
        # per-queue ORDER. First-matmul deps go first.
        # gpsimd carries NO bulk inputs: its first compute op after DMA
        # triggers forces a pool reconfig that waits for ALL its queued
        # transfers to drain — the preamble below (which gates all of
        # attention: tri mask, bias broadcasts) would stall ~25us.
        qs = [nc.sync, nc.scalar]
        nc.sync.dma_start(bqk_sb[:], bqk)
        bv_row = att.tile([1, CL], F32, tag="brow", bufs=2, name="bv_row")
        nc.sync.dma_start(bv_row[:], bv.rearrange("(o c) -> o c", o=1))
        bp_row = att.tile([1, CL], F32, tag="brow", bufs=2, name="bp_row")
        nc.scalar.dma_start(bp_row[:], bp.rearrange("(o c) -> o c", o=1))

        # gpsimd preamble right away (waits only on the two bias rows)
        nc.gpsimd.partition_broadcast(bv_bc[:], bv_row[:])
        nc.gpsimd.partition_broadcast(bp_bc[:], bp_row[:])
        nc.gpsimd.memset(ones_f[:], 1.0)
        nc.gpsimd.memset(tri[:], 1.0)
        nc.gpsimd.affine_select(
            out=tri[:], in_=tri[:], compare_op=mybir.AluOpType.is_ge,
            fill=0.0, base=0, channel_multiplier=-1, pattern=[[1, P]])

        # Early bulk (needed in the first ~35us) rides all 3 queues —
        # gpsimd's transfers here all complete before its first
        # normalize broadcast (~50us), so the pool-reconfig drain-wait
        # is a no-op. Late bulk (x q2/q3, wp) stays OFF gpsimd.
        q3_ = [nc.sync, nc.scalar, nc.gpsimd]
        hc = n_cc // 2
        nc.sync.dma_start(wqk_sb[0][:, 0:hc], wqk[0, :, 0:hc])
        nc.gpsimd.dma_start(wqk_sb[0][:, hc:], wqk[0, :, hc:])
        nc.scalar.dma_start(wqk_sb[n_jt][:, 0:hc], wqk[n_jt, :, 0:hc])
        nc.gpsimd.dma_start(wqk_sb[n_jt][:, hc:], wqk[n_jt, :, hc:])
        # x quarter 0 striped
        ri = 0
        for c in range(n_cc):
            q3_[ri % 3].dma_start(x_sb[c][:, 0:QT],
                                  xT[c * P:(c + 1) * P, 0:QT])
            ri += 1
        # wv quarters next (v0-3 pop inside tile (0,0); its matmuls
        # consume wv c-chunk by c-chunk as these land)
        nc.sync.dma_start(wv_sb[:, 0:2], wv_s[:, 0:2])
        nc.scalar.dma_start(wv_sb[:, 2:4], wv_s[:, 2:4])
        nc.gpsimd.dma_start(wv_sb[:, 4:6], wv_s[:, 4:6])
        nc.sync.dma_start(wv_sb[:, 6:8], wv_s[:, 6:8])
        # wqk f=1,5: round-0 fillers qk(f, 0) for pairs 1-2 pop early and
        # their matmuls head-of-line-block the PE queue until these land
        nc.scalar.dma_start(wqk_sb[1][:], wqk[1])
        nc.gpsimd.dma_start(wqk_sb[5][:], wqk[5])
        for f in (2, 6, 3, 7):
            q3_[ri % 3].dma_start(wqk_sb[f][:], wqk[f])
            ri += 1
        # x quarter 1 still on all three (needed ~30us)
        for c in range(n_cc):
            q3_[ri % 3].dma_start(x_sb[c][:, QT:2 * QT],
                                  xT[c * P:(c + 1) * P, QT:2 * QT])
            ri += 1
        # late bulk: sync+scalar only
        for q4 in range(2, 4):
            for c in range(n_cc):
                qs[ri % 2].dma_start(
                    x_sb[c][:, q4 * QT:(q4 + 1) * QT],
                    xT[c * P:(c + 1) * P, q4 * QT:(q4 + 1) * QT])
                ri += 1
            if q4 == 2:
                nc.scalar.dma_start(wp_sb[:, 0:hc], wp_s[:, 0:hc])
                nc.sync.dma_start(wp_sb[:, hc:], wp_s[:, hc:])

        # ---- AG segments ---------------------------------------------
        # per pair: one T-half segment for j=0/1, quarter segments for
        # j=2 and j=3 -> 12 AllGathers, staggered across rounds.
        segs = {}   # (pr, j) -> [ti, to, col_base, j_set]
        for pr in range(n_pair):
            ti = dram.tile([P, TH], BF16, tag=f"agi{pr}_h0", name=f"agi{pr}_h0")
            to = dram.tile([2, P, TH], BF16, tag=f"ago{pr}_h0",
                           name=f"ago{pr}_h0")
            for j in (0, 1):
                segs[(pr, j)] = [ti, to, j * QT, {0, 1}]
            for j in (2, 3):
                ti = dram.tile([P, QT], BF16, tag=f"agi{pr}_q{j}",
                               name=f"agi{pr}_q{j}")
                to = dram.tile([2, P, QT], BF16, tag=f"ago{pr}_q{j}",
                               name=f"ago{pr}_q{j}")
                segs[(pr, j)] = [ti, to, 0, {j}]

        # ---- compute atoms -------------------------------------------
        def v_atom(t):
            """V for t-chunk t: [128 t, CL] + bias, ones col per head."""
            pv = ps.tile([P, CL], F32, tag="qps", bufs=2, name="pv")
            for c in range(n_cc):
                nc.tensor.matmul(
                    pv[:], x_sb[c][:, t * KC:(t + 1) * KC], wv_sb[:, c, :],
                    start=(c == 0), stop=(c == n_cc - 1))
            nc.vector.tensor_copy(v_sb[t][:, :, HS:HS + 1], ones_f[:])
            nc.vector.tensor_add(
                v_sb[t][:, :, 0:HS],
                pv.rearrange("p (h e) -> p h e", e=HS),
                bv_bc.rearrange("p (h e) -> p h e", e=HS))

        def qk_atom(f, t):
            """q/k f-tile x one t-tile of 512: 8 matmuls + bias to SBUF."""
            pq = ps.tile([P, QT], F32, tag="qps", bufs=2, name="pq")
            for c in range(n_cc):
                nc.tensor.matmul(
                    pq[:], wqk_sb[f][:, c, :],
                    x_sb[c][:, t * QT:(t + 1) * QT],
                    start=(c == 0), stop=(c == n_cc - 1))
            nc.vector.tensor_scalar_add(
                qk_sb[f][:, t * QT:(t + 1) * QT], pq[:], bqk_sb[:, f:f + 1])

        # yr[(pr, j, gp)] = SBUF tile with replica gp's y^T block for
        # q-tile j of pair pr ([128 feat, 512 q], fetched post-AG).
        # Fetches ride the filler queue (gated a couple of positions
        # after their AG) so their AG-completion wait never head-of-line
        # blocks a latency-critical engine queue; sync hosts them (the
        # gpsimd queue must stay clear for normalize broadcasts and AG
        # triggers, scalar for the exp chain).
        yr = {}

        def yr_fetch(pr, j):
            _, to, col_base, _ = segs[(pr, j)]
            for gp in range(2):
                t_ = att.tile([P, QT], BF16, tag="yr", bufs=32, name="yr")
                nc.sync.dma_start(t_[:], to[gp, :, col_base:col_base + QT])
                yr[(pr, j, gp)] = t_

        oc_t = {}

        def proj_main(t):
            """Projection for t-chunk t, pairs 0-2: 6 matmuls + bias add.

            The pr 3 group is a separate atom (proj_tail): pair 3's AG
            is always the last to land; one 8-matmul group would hold
            its PSUM bank for the whole wait, and in the drain its
            waiting matmuls would FIFO-block ready ones behind them.
            """
            j = t // KPQ
            col = (t % KPQ) * P
            oc = att.tile([P, CL], F32, tag="oacc", bufs=8, name="oacc")
            oc_t[t] = oc
            po = ps.tile([P, CL], F32, tag="qps", bufs=2, name="po")
            k = 0
            for pr in range(n_pair - 1):
                for gp in range(2):
                    nc.tensor.matmul(
                        po[:], yr[(pr, j, gp)][:, col:col + P],
                        wp_sb[:, gp * n_pair + pr, :],
                        start=(k == 0), stop=(k == 2 * (n_pair - 1) - 1))
                    k += 1
            nc.vector.tensor_add(oc[:], po[:], bp_bc[:])

        def proj_tail(t):
            j = t // KPQ
            col = (t % KPQ) * P
            oc = oc_t[t]
            po3 = ps.tile([P, CL], F32, tag="qps", bufs=2, name="po3")
            for gp in range(2):
                nc.tensor.matmul(
                    po3[:], yr[(n_pair - 1, j, gp)][:, col:col + P],
                    wp_sb[:, gp * n_pair + n_pair - 1, :],
                    start=(gp == 0), stop=(gp == 1))
            nc.vector.tensor_add(oc[:], oc[:], po3[:])
            # keep outputs off sync mid-kernel (ti writes + fetches live
            # there); scalar only at the drain, when exp is done
            oq = nc.scalar if t >= 12 else (nc.gpsimd if t % 2 else nc.sync)
            oq.dma_start(out_ext[t * P:(t + 1) * P, :], oc[:])

        def proj_atom(t):
            proj_main(t)
            proj_tail(t)

        # ---- filler queue --------------------------------------------
        # (min_pos, thunk): position = j*4 + pr of the attention tile at
        # or after which the atom may be emitted.
        filler = []
        # v0-3 ride the filler queue (popped inside tile (0,0) after its
        # first score matmuls): emitted upfront, their late-arriving wv
        # DMA head-of-line-blocks the PE queue before attention can start
        for t in range(4):
            filler.append((0, lambda t=t: v_atom(t)))
        for f in (1, 5):
            filler.append((0, lambda f=f: qk_atom(f, 0)))
        for f in (2, 6):
            filler.append((1, lambda f=f: qk_atom(f, 0)))
        for f in (3, 7):
            filler.append((1, lambda f=f: qk_atom(f, 0)))
        # round r+1 deps staged across the later tiles of round r (the
        # x quarter r+1 DMAs land mid-round; popping these too early
        # head-of-line-blocks the PE queue on the DMA semaphore).
        # v t-chunks 8-15 are only consumed by chunk i>=8 of their round,
        # so they slide INTO rounds 2/3 as PE filler for the exp-bound
        # stretches there (all four must pop within the round's FIRST
        # tile, which itself consumes them at chunks i>=8 / 12).
        # v4-7 and the qk t=1 batch wait for x quarter 1 (~33us): gate
        # them at pos 3 so their pops don't block the PE queue earlier
        for k in range(4):
            filler.append((3, lambda t=4 + k: v_atom(t)))
        for k, f in enumerate((0, 4, 1, 5, 2, 6, 3, 7)):
            filler.append((2 + k // 4, lambda f=f: qk_atom(f, 1)))
        for k in range(4):
            filler.append((8, lambda t=8 + k: v_atom(t)))
        for k in range(4):
            filler.append((12, lambda t=12 + k: v_atom(t)))
        for r in range(1, 3):
            t = r + 1
            for k, f in enumerate((0, 4, 1, 5, 2, 6, 3, 7)):
                filler.append((4 * r + 2 + k // 4,
                               lambda f=f, t=t: qk_atom(f, t)))
        # yr fetches: gated well after their AG fires — AG *execution*
        # lags its trigger by up to ~25us (CC-stream serialization plus
        # inter-core skew: the collective starts only when BOTH cores of
        # the pair arrive). A fetch popped before its AG finished would
        # head-of-line-block sync, delaying later ti writes and
        # cascading into the q3 AG chain.
        for pr in range(n_pair):
            filler.append((10 + pr, lambda pr=pr: yr_fetch(pr, 1)))
            filler.append((10 + pr, lambda pr=pr: yr_fetch(pr, 0)))
        for pr, g in ((0, 13), (1, 14), (2, 14)):
            filler.append((g, lambda pr=pr: yr_fetch(pr, 2)))
        # projection: h0 t-chunks fill late round 3 (the exp-bound
        # stretch with the least native PE work); q2/q3 drain in a
        # hand-ordered sequence: fetches whose AGs are long done, then
        # every ready matmul group (they overlap the final AG), then the
        # pr3 groups that must wait for it.
        for t in range(8):
            filler.append((14 if t < 4 else 15, lambda t=t: proj_atom(t)))
        filler.append((16, lambda: yr_fetch(3, 2)))
        for pr in range(n_pair):
            filler.append((16, lambda pr=pr: yr_fetch(pr, 3)))
        for t in range(8, 16):
            filler.append((16, lambda t=t: proj_main(t)))
        for t in range(8, 16):
            filler.append((16, lambda t=t: proj_tail(t)))

        def pop_filler(pos):
            for idx, (mp, thunk) in enumerate(filler):
                if mp <= pos:
                    filler.pop(idx)
                    thunk()
                    return True
            return False

        # ---- attention tile ------------------------------------------
        def att_tile(pr, j, pos):
            """Both heads of pair pr on q-tile j; 2-deep st -> exp -> av."""
            kT = qk_sb[n_pair + pr]
            qTt = qk_sb[pr]
            yps = {rr: ps.tile([P, QT], F32, tag=f"yp{rr}", bufs=1,
                               name=f"yp{rr}") for rr in range(2)}
            imax = KPQ * j + KPQ
            if pos > 0:
                pop_filler(pos)
                pop_filler(pos)
                pop_filler(pos)
            pend = []   # pipelined (i, off, pt) awaiting av

            def av(iv, offv, ptv):
                for rr in range(2):
                    nc.tensor.matmul(
                        yps[rr][0:HS + 1, offv:QT],
                        v_sb[iv][:, 2 * pr + rr, 0:HS + 1],
                        ptv[:, rr, offv:QT],
                        start=(iv == 0), stop=(iv == imax - 1))

            for i in range(imax):
                diag = (i // KPQ == j)
                # causally trim diagonal chunks to q >= i*KC
                off = KC * (i % KPQ) if diag else 0
                st = ps.tile([P, 2, QT], F32, tag="st", bufs=2, name="st")
                for rr in range(2):
                    ro = HS * rr
                    nc.tensor.matmul(
                        st[:, rr, off:QT],
                        kT[ro:ro + HS, i * KC:(i + 1) * KC],
                        qTt[ro:ro + HS, j * QT + off:(j + 1) * QT],
                        start=True, stop=True)
                pt = att.tile([P, 2, QT], BF16, tag="pt", bufs=4, name="pt")
                nc.scalar.activation(
                    pt[:, :, off:QT], st[:, :, off:QT],
                    mybir.ActivationFunctionType.Exp, scale=scale)
                if diag:
                    for rr in range(2):
                        # zero above the diagonal in the leading 128x128
                        # triangle, in place
                        nc.vector.tensor_mul(
                            pt[:, rr, off:off + KC],
                            pt[:, rr, off:off + KC], tri[:])
                pend.append((i, off, pt))
                if len(pend) > 2:
                    av(*pend.pop(0))
                if i % 2 == 1 or pos == 0:
                    pop_filler(pos)
            while pend:
                av(*pend.pop(0))

            # normalize both heads into one y2 tile (rr=1 write is
            # partition-shifted 0->64; validated on HW)
            y2 = att.tile([P, QT], BF16, tag="y2", bufs=3, name="y2")
            for rr in range(2):
                # keep this chain at exactly these 4 ops (see docstring)
                row = att.tile([1, QT], F32, tag="row", bufs=3, name="row")
                nc.vector.tensor_copy(row[:], yps[rr][HS:HS + 1, :])
                rec = att.tile([1, QT], F32, tag="rec", bufs=3, name="rec")
                nc.vector.reciprocal_approx_fast(rec[:], row[:])
                rb = att.tile([HS, QT], F32, tag="rb", bufs=3, name="rb")
                nc.gpsimd.partition_broadcast(rb[:], rec[:])
                nc.vector.tensor_mul(y2[rr * HS:(rr + 1) * HS, :],
                                     yps[rr][0:HS, :], rb[:])
            ti, to, col_base, j_set = segs[(pr, j)]
            nc.sync.dma_start(ti[:, col_base:col_base + QT], y2[:])
            if j == max(j_set):
                nc.gpsimd.collective_compute(
                    "AllGather", mybir.AluOpType.bypass,
                    replica_groups=PAIRS,
                    ins=[ti.opt()], outs=[to.opt()])

        # ---- schedule ------------------------------------------------
        # minimal upfront: what round 0 pair 0 needs; everything else
        # flows in through the filler queue
        qk_atom(0, 0)
        qk_atom(n_jt, 0)
        for t in range(4):
            v_atom(t)

        for j in range(n_jt):
            for pr in range(n_pair):
                att_tile(pr, j, j * n_pair + pr)
        while pop_filler(4 * n_jt):
            pass

    nc.compile()
    return nc


def shard_inputs(x, w_attn, b_attn, w_proj, b_proj):
    """Slice/transpose/shuffle full inputs into 8 per-core input maps."""
    Bq, T, C = x.shape
    CL = C // 2
    n_cc = C // P
    n_f = 2 * CL // P
    bf = ml_dtypes.bfloat16
    in_maps = []
    for i in range(N_CORES):
        b, g = i // 2, i % 2
        sl = slice(CL * g, CL * (g + 1))
        wq = w_attn[:, sl]
        wk = w_attn[:, C + CL * g:C + CL * (g + 1)]
        wvv = w_attn[:, 2 * C + CL * g:2 * C + CL * (g + 1)]
        wqk = np.concatenate([wq, wk], axis=1)          # [C, 2CL]
        # [C, 2CL] -> [f, p, c, m]: row r = c*128+p, col = f*128+m
        wqk_s = np.ascontiguousarray(
            wqk.reshape(n_cc, P, n_f, P).transpose(2, 1, 0, 3)).astype(bf)
        wv_shuf = np.ascontiguousarray(
            wvv.reshape(n_cc, P, CL).transpose(1, 0, 2)).astype(bf)
        wp_shuf = np.ascontiguousarray(
            w_proj[:, sl].reshape(n_cc, P, CL).transpose(1, 0, 2)).astype(bf)
        in_maps.append({
            "xT": np.ascontiguousarray(x[b].T).astype(bf),
            "wqk": wqk_s,
            "wv_s": wv_shuf,
            "wp_s": wp_shuf,
            "bqk": np.ascontiguousarray(
                np.concatenate([b_attn[sl],
                                b_attn[C + CL * g:C + CL * (g + 1)]])
                .reshape(n_f, P).T),
            "bv": np.ascontiguousarray(b_attn[2 * C + CL * g:2 * C + CL * (g + 1)]),
            "bp": np.ascontiguousarray(b_proj[sl]),
        })
    return in_maps


def gather_outputs(results, B, T, C):
    CL = C // 2
    out = np.empty((B, T, C), dtype=np.float32)
    for i in range(N_CORES):
        b, g = i // 2, i % 2
        out[b, :, CL * g:CL * (g + 1)] = results[i]["out"]
    return out


_NC_CACHE = {}


def get_nc(T, C):
    key = (T, C)
    if key not in _NC_CACHE:
        _NC_CACHE[key] = build_nc(T=T, C=C, HL=C // HS // 2)
    return _NC_CACHE[key]


def kernel(x, w_attn, b_attn, w_proj, b_proj):
    x = np.asarray(x, dtype=np.float32)
    w_attn = np.asarray(w_attn, dtype=np.float32)
    b_attn = np.asarray(b_attn, dtype=np.float32)
    w_proj = np.asarray(w_proj, dtype=np.float32)
    b_proj = np.asarray(b_proj, dtype=np.float32)

    Bq, T, C = x.shape
    nc = get_nc(T, C)

    in_maps = shard_inputs(x, w_attn, b_attn, w_proj, b_proj)
    trace = os.environ.get("KERNEL_TRACE", "0") == "1"
    res = bass_utils.run_bass_kernel_spmd(
        nc, in_maps, core_ids=list(range(N_CORES)), trace=trace)
    if trace and res.exec_time_ns is not None:
        print(f"HW exec time: {res.exec_time_ns} ns", flush=True)
        kernel.last_exec_time_ns = res.exec_time_ns
        kernel.last_results = res
    return gather_outputs(res.results, Bq, T, C)


# revision 34
# speedup vs baseline: 1.0424x; 1.0193x over previous
"""Causal self-attention (B=4, T=2048, C=1024, H=16) on 8 TRN2 NeuronCores.

Sharding: data-parallel on batch (4) x tensor-parallel on heads (2 groups of
8). Core i handles batch i//2 and head-group i%2.

Schedule: j-OUTER rounds — for each q-tile j (512 wide), all 4 head-pairs
attend in sequence. This staggers the pairwise AllGathers across the whole
kernel instead of stacking them at the tail (the prior pair-outer schedule
spent its last ~50us at half HAM clock waiting on the final pair's AG chain).

Per (pair, j) tile: scores^T [k, q] stripes, 2-deep pipelined st -> exp ->
av; causal diagonal blocks N-trimmed + 128x128 triangle masked by DVE mul
with a 0/1 mask. exp is safe without max subtraction (scores*hs^-0.5 are
O(+-10)). sum(exp) rides as a ones column in v (av out M=65). The two
heads' score matmuls land on PE row tiles (0,0)/(64,0) and run concurrent.

Normalize: per head the exact 4-op chain (stage row to base-0 SBUF, fast
reciprocal, gpsimd partition_broadcast, mul) — do not add ops; a 5-op chain
collapses the HW clock. rr=0 mul writes y2[0:64], rr=1 writes y2[64:128]
(partition-shifted DVE write, HW-validated), then ONE dma lands y2 in the
AG input segment.

Collectives: 12 pairwise AllGathers — per pair one T-half segment for
j=0/1 and quarter segments for j=2, j=3. After each AG, yr tiles ([128,512]
per (pair, j, group), 1KB DMA lines) are fetched to SBUF; projection for
t-chunk t accumulates 8 matmuls (4 pairs x 2 groups) into one PSUM bank,
one DVE add folds b_proj, out DMA rotates across queues. Output lands
spread over the second half of the kernel instead of all at the end.

Startup: input DMAs priority-ordered across 5 trigger queues (sync, gpsimd,
scalar, vector, tensor): first-matmul deps (bqk, wqk[0]/wqk[4] halves,
x quarter 0) first, then wv, then the rest in consumption order.

dtypes: matmul operands bf16, accumulation fp32 in PSUM, softmax
normalization fp32. (fp8 anywhere fails the 2e-2 gate: ~6% y error.)

HW gotchas (CoreSim passes these; only real HW fails):
  - a single 65-partition DVE copy from PSUM silently corrupts data
  - gpsimd custom-DVE reciprocal misreads PSUM and misreads inputs at a
    nonzero base partition (stage rows into base-0 SBUF first)
  - junk "heater" matmuls and 16x fine-grained AllGathers both regress
"""

import os
import sys
from contextlib import ExitStack

import numpy as np
import ml_dtypes

if "/opt/trn_rl_repo" not in sys.path:
    sys.path.insert(0, "/opt/trn_rl_repo")

import concourse.bass as bass
import concourse.mybir as mybir
import concourse.tile as tile
from concourse import bacc
from concourse import bass_utils

F32 = mybir.dt.float32
BF16 = mybir.dt.bfloat16
P = 128          # SBUF partitions
QT = 512         # q tile (matmul free dim)
KC = 128         # k chunk (psum partition dim)
HS = 64          # head size
KPQ = QT // KC   # k chunks per q tile

N_CORES = 8
PAIRS = [[0, 1], [2, 3], [4, 5], [6, 7]]

B_FULL, T_FULL, C_FULL, H_FULL = 4, 2048, 1024, 16


def build_nc(T=T_FULL, C=C_FULL, HL=H_FULL // 2):
    """Build the SPMD graph for one core (all 8 cores run the same graph).

    Per-core input tensors:
      xT    [C, T] bf16       x[b] transposed
      wqk   [2CL/P, P, C/P, P] bf16  w_attn q|k cols, host-shuffled [f,p,c,m]
      wv_s  [P, C/P, CL] bf16 w_attn v cols, host-shuffled [p,c,m]
      wp_s  [P, C/P, CL] bf16 w_proj (all 1024 rows as 2 groups x 4 pairs,
                              this core's 512 out cols), host-shuffled
      bqk   [P, 2*CL/P] f32, bv [CL] f32, bp [CL] f32
    Output: out [T, CL] f32.
    """
    CL = HL * HS                 # local width (q, k, v, out-cols each)
    n_cc = C // P                # x feature chunks (8)
    n_f = 2 * CL // P            # q|k f-tiles (4 q then 4 k)
    n_jt = T // QT               # q tiles / rounds (4)
    n_kt = T // KC               # k chunks / v t-chunks (16)
    n_pair = HL // 2             # head pairs (4)
    TH = T // 2
    scale = HS ** -0.5

    nc = bacc.Bacc("TRN2", target_bir_lowering=False, debug=False,
                   num_devices=N_CORES)

    xT = nc.dram_tensor("xT", [C, T], BF16, kind="ExternalInput").ap()
    wqk = nc.dram_tensor("wqk", [n_f, P, n_cc, P], BF16,
                         kind="ExternalInput").ap()
    wv_s = nc.dram_tensor("wv_s", [P, n_cc, CL], BF16,
                          kind="ExternalInput").ap()
    wp_s = nc.dram_tensor("wp_s", [P, n_cc, CL], BF16,
                          kind="ExternalInput").ap()
    bqk = nc.dram_tensor("bqk", [P, 2 * CL // P], F32,
                         kind="ExternalInput").ap()
    bv = nc.dram_tensor("bv", [CL], F32, kind="ExternalInput").ap()
    bp = nc.dram_tensor("bp", [CL], F32, kind="ExternalInput").ap()
    out_ext = nc.dram_tensor("out", [T, CL], F32, kind="ExternalOutput").ap()

    with ExitStack() as ctx:
        tc = ctx.enter_context(tile.TileContext(nc))

        persist = ctx.enter_context(tc.tile_pool(name="persist", bufs=1))
        dram = ctx.enter_context(tc.tile_pool(name="dram", bufs=1, space="DRAM"))
        # st 2x2 banks + yp0 + yp1 + qps 2 = 8 banks
        ps = ctx.enter_context(tc.tile_pool(name="ps", bufs=1, space="PSUM"))
        att = ctx.enter_context(tc.tile_pool(name="att", bufs=1))

        # ---- persistent SBUF tiles -----------------------------------
        wqk_sb = [persist.tile([P, n_cc, P], BF16, tag=f"wqk{f}",
                               name=f"wqk{f}") for f in range(n_f)]
        wv_sb = persist.tile([P, n_cc, CL], BF16, tag="wv", name="wv")
        wp_sb = persist.tile([P, n_cc, CL], BF16, tag="wp", name="wp")
        x_sb = [persist.tile([P, T], BF16, tag=f"x{c}", name=f"x{c}")
                for c in range(n_cc)]
        qk_sb = [persist.tile([P, T], BF16, tag=f"qk{f}", name=f"qk{f}")
                 for f in range(n_f)]
        v_sb = [persist.tile([P, HL, HS + 2], BF16, tag=f"v{t}",
                             name=f"v{t}") for t in range(n_kt)]
        bqk_sb = persist.tile([P, n_f], F32, tag="bqk", name="bqk_sb")
        bv_bc = persist.tile([P, CL], F32, tag="bv_bc", name="bv_bc")
        bp_bc = persist.tile([P, CL], F32, tag="bp_bc", name="bp_bc")
        ones_f = persist.tile([P, HL, 1], F32, tag="ones_f", name="ones_f")
        # tri[p, g] = 1 where g >= p else 0 (keep-at-or-above-diagonal)
        tri = persist.tile([P, P], BF16, tag="tri", name="tri")

        # ---- input DMAs: priority order, sync+scalar queues only ------
        # (only sync/gpsimd/scalar can initiate DMAs.) Every logical
        # queue sprays across all 16 DMA engines; what matters is
        # per-queue ORDER. First-matmul deps go first.
        # gpsimd carries NO bulk inputs: its first compute op after DMA
        # triggers forces a pool reconfig that waits for ALL its queued
        # transfers to drain — the preamble below (which gates all of
        # attention: tri mask, bias broadcasts) would stall ~25us.
        qs = [nc.sync, nc.scalar]
        nc.sync.dma_start(bqk_sb[:], bqk)
        bv_row = att.tile([1, CL], F32, tag="brow", bufs=2, name="bv_row")
        nc.sync.dma_start(bv_row[:], bv.rearrange("(o c) -> o c", o=1))
        bp_row = att.tile([1, CL], F32, tag="brow", bufs=2, name="bp_row")
        nc.scalar.dma_start(bp_row[:], bp.rearrange("(o c) -> o c", o=1))

        # gpsimd preamble right away (waits only on the two bias rows)
        nc.gpsimd.partition_broadcast(bv_bc[:], bv_row[:])
        nc.gpsimd.partition_broadcast(bp_bc[:], bp_row[:])
        nc.gpsimd.memset(ones_f[:], 1.0)
        nc.gpsimd.memset(tri[:], 1.0)
        nc.gpsimd.affine_select(
            out=tri[:], in_=tri[:], compare_op=mybir.AluOpType.is_ge,
            fill=0.0, base=0, channel_multiplier=-1, pattern=[[1, P]])

        # Early bulk (needed in the first ~35us) rides all 3 queues —
        # gpsimd's transfers here all complete before its first
        # normalize broadcast (~50us), so the pool-reconfig drain-wait
        # is a no-op. Late bulk (x q2/q3, wp) stays OFF gpsimd.
        q3_ = [nc.sync, nc.scalar, nc.gpsimd]
        hc = n_cc // 2
        nc.sync.dma_start(wqk_sb[0][:, 0:hc], wqk[0, :, 0:hc])
        nc.gpsimd.dma_start(wqk_sb[0][:, hc:], wqk[0, :, hc:])
        nc.scalar.dma_start(wqk_sb[n_jt][:, 0:hc], wqk[n_jt, :, 0:hc])
        nc.gpsimd.dma_start(wqk_sb[n_jt][:, hc:], wqk[n_jt, :, hc:])
        # x quarter 0 striped
        ri = 0
        for c in range(n_cc):
            q3_[ri % 3].dma_start(x_sb[c][:, 0:QT],
                                  xT[c * P:(c + 1) * P, 0:QT])
            ri += 1
        # wv quarters next (v0-3 pop inside tile (0,0); its matmuls
        # consume wv c-chunk by c-chunk as these land)
        nc.sync.dma_start(wv_sb[:, 0:2], wv_s[:, 0:2])
        nc.scalar.dma_start(wv_sb[:, 2:4], wv_s[:, 2:4])
        nc.gpsimd.dma_start(wv_sb[:, 4:6], wv_s[:, 4:6])
        nc.sync.dma_start(wv_sb[:, 6:8], wv_s[:, 6:8])
        # wqk f=1,5: round-0 fillers qk(f, 0) for pairs 1-2 pop early and
        # their matmuls head-of-line-block the PE queue until these land
        nc.scalar.dma_start(wqk_sb[1][:], wqk[1])
        nc.gpsimd.dma_start(wqk_sb[5][:], wqk[5])
        for f in (2, 6, 3, 7):
            q3_[ri % 3].dma_start(wqk_sb[f][:], wqk[f])
            ri += 1
        # x quarter 1 still on all three (needed ~30us)
        for c in range(n_cc):
            q3_[ri % 3].dma_start(x_sb[c][:, QT:2 * QT],
                                  xT[c * P:(c + 1) * P, QT:2 * QT])
            ri += 1
        # late bulk: sync+scalar only
        for q4 in range(2, 4):
            for c in range(n_cc):
                qs[ri % 2].dma_start(
                    x_sb[c][:, q4 * QT:(q4 + 1) * QT],
                    xT[c * P:(c + 1) * P, q4 * QT:(q4 + 1) * QT])
                ri += 1
            if q4 == 2:
                nc.scalar.dma_start(wp_sb[:, 0:hc], wp_s[:, 0:hc])
                nc.sync.dma_start(wp_sb[:, hc:], wp_s[:, hc:])

        # ---- AG segments ---------------------------------------------
        # per pair: one T-half segment for j=0/1, quarter segments for
        # j=2 and j=3 -> 12 AllGathers, staggered across rounds.
        segs = {}   # (pr, j) -> [ti, to, col_base, j_set]
        for pr in range(n_pair):
            ti = dram.tile([P, TH], BF16, tag=f"agi{pr}_h0", name=f"agi{pr}_h0")
            to = dram.tile([2, P, TH], BF16, tag=f"ago{pr}_h0",
                           name=f"ago{pr}_h0")
            for j in (0, 1):
                segs[(pr, j)] = [ti, to, j * QT, {0, 1}]
            for j in (2, 3):
                ti = dram.tile([P, QT], BF16, tag=f"agi{pr}_q{j}",
                               name=f"agi{pr}_q{j}")
                to = dram.tile([2, P, QT], BF16, tag=f"ago{pr}_q{j}",
                               name=f"ago{pr}_q{j}")
                segs[(pr, j)] = [ti, to, 0, {j}]

        # ---- compute atoms -------------------------------------------
        def v_atom(t):
            """V for t-chunk t: [128 t, CL] + bias, ones col per head."""
            pv = ps.tile([P, CL], F32, tag="qps", bufs=2, name="pv")
            for c in range(n_cc):
                nc.tensor.matmul(
                    pv[:], x_sb[c][:, t * KC:(t + 1) * KC], wv_sb[:, c, :],
                    start=(c == 0), stop=(c == n_cc - 1))
            nc.vector.tensor_copy(v_sb[t][:, :, HS:HS + 1], ones_f[:])
            nc.vector.tensor_add(
                v_sb[t][:, :, 0:HS],
                pv.rearrange("p (h e) -> p h e", e=HS),
                bv_bc.rearrange("p (h e) -> p h e", e=HS))

        def qk_atom(f, t):
            """q/k f-tile x one t-tile of 512: 8 matmuls + bias to SBUF."""
            pq = ps.tile([P, QT], F32, tag="qps", bufs=2, name="pq")
            for c in range(n_cc):
                nc.tensor.matmul(
                    pq[:], wqk_sb[f][:, c, :],
                    x_sb[c][:, t * QT:(t + 1) * QT],
                    start=(c == 0), stop=(c == n_cc - 1))
            nc.vector.tensor_scalar_add(
                qk_sb[f][:, t * QT:(t + 1) * QT], pq[:], bqk_sb[:, f:f + 1])

        # yr[(pr, j, gp)] = SBUF tile with replica gp's y^T block for
        # q-tile j of pair pr ([128 feat, 512 q], fetched post-AG).
        # Fetches ride the filler queue (gated a couple of positions
        # after their AG) so their AG-completion wait never head-of-line
        # blocks a latency-critical engine queue; sync hosts them (the
        # gpsimd queue must stay clear for normalize broadcasts and AG
        # triggers, scalar for the exp chain).
        yr = {}

        def yr_fetch(pr, j):
            _, to, col_base, _ = segs[(pr, j)]
            for gp in range(2):
                t_ = att.tile([P, QT], BF16, tag="yr", bufs=32, name="yr")
                nc.sync.dma_start(t_[:], to[gp, :, col_base:col_base + QT])
                yr[(pr, j, gp)] = t_

        oc_t = {}

        def proj_main(t):
            """Projection for t-chunk t, pairs 0-2: 6 matmuls + bias add.

            The pr 3 group is a separate atom (proj_tail): pair 3's AG
            is always the last to land; one 8-matmul group would hold
            its PSUM bank for the whole wait, and in the drain its
            waiting matmuls would FIFO-block ready ones behind them.
            """
            j = t // KPQ
            col = (t % KPQ) * P
            oc = att.tile([P, CL], F32, tag="oacc", bufs=8, name="oacc")
            oc_t[t] = oc
            po = ps.tile([P, CL], F32, tag="qps", bufs=2, name="po")
            k = 0
            for pr in range(n_pair - 1):
                for gp in range(2):
                    nc.tensor.matmul(
                        po[:], yr[(pr, j, gp)][:, col:col + P],
                        wp_sb[:, gp * n_pair + pr, :],
                        start=(k == 0), stop=(k == 2 * (n_pair - 1) - 1))
                    k += 1
            nc.vector.tensor_add(oc[:], po[:], bp_bc[:])

        def proj_tail(t):
            j = t // KPQ
            col = (t % KPQ) * P
            oc = oc_t[t]
            po3 = ps.tile([P, CL], F32, tag="qps", bufs=2, name="po3")
            for gp in range(2):
                nc.tensor.matmul(
                    po3[:], yr[(n_pair - 1, j, gp)][:, col:col + P],
                    wp_sb[:, gp * n_pair + n_pair - 1, :],
                    start=(gp == 0), stop=(gp == 1))
            nc.vector.tensor_add(oc[:], oc[:], po3[:])
            # keep outputs off sync mid-kernel (ti writes + fetches live
            # there); scalar only at the drain, when exp is done
            oq = nc.scalar if t >= 12 else (nc.gpsimd if t % 2 else nc.sync)
            oq.dma_start(out_ext[t * P:(t + 1) * P, :], oc[:])

        def proj_atom(t):
            proj_main(t)
            proj_tail(t)

        # ---- filler queue --------------------------------------------
        # (min_pos, thunk): position = j*4 + pr of the attention tile at
        # or after which the atom may be emitted.
        filler = []
        # v0-3 ride the filler queue (popped inside tile (0,0) after its
        # first score matmuls): emitted upfront, their late-arriving wv
        # DMA head-of-line-blocks the PE queue before attention can start
        for t in range(4):
            filler.append((0, lambda t=t: v_atom(t)))
        for f in (1, 5):
            filler.append((0, lambda f=f: qk_atom(f, 0)))
        for f in (2, 6):
            filler.append((1, lambda f=f: qk_atom(f, 0)))
        for f in (3, 7):
            filler.append((1, lambda f=f: qk_atom(f, 0)))
        # round r+1 deps staged across the later tiles of round r (the
        # x quarter r+1 DMAs land mid-round; popping these too early
        # head-of-line-blocks the PE queue on the DMA semaphore).
        # v t-chunks 8-15 are only consumed by chunk i>=8 of their round,
        # so they slide INTO rounds 2/3 as PE filler for the exp-bound
        # stretches there (all four must pop within the round's FIRST
        # tile, which itself consumes them at chunks i>=8 / 12).
        # v4-7 and the qk t=1 batch wait for x quarter 1 (~33us): gate
        # them at pos 3 so their pops don't block the PE queue earlier
        for k in range(4):
            filler.append((3, lambda t=4 + k: v_atom(t)))
        for k, f in enumerate((0, 4, 1, 5, 2, 6, 3, 7)):
            filler.append((2 + k // 4, lambda f=f: qk_atom(f, 1)))
        for k in range(4):
            filler.append((8, lambda t=8 + k: v_atom(t)))
        for k in range(4):
            filler.append((12, lambda t=12 + k: v_atom(t)))
        for k, f in enumerate((0, 4, 1, 5, 2, 6, 3, 7)):
            filler.append((6 + k // 4, lambda f=f: qk_atom(f, 2)))
        # qk t=3 spread from round 2's start: its early tiles starve
        # (exp-bound, v8-11 spent, proj not yet eligible)
        for k, f in enumerate((0, 4, 1, 5, 2, 6, 3, 7)):
            filler.append((8 + k // 2, lambda f=f: qk_atom(f, 3)))
        # yr fetches: gated well after their AG fires — AG *execution*
        # lags its trigger by up to ~25us (CC-stream serialization plus
        # inter-core skew: the collective starts only when BOTH cores of
        # the pair arrive). A fetch popped before its AG finished would
        # head-of-line-block sync, delaying later ti writes and
        # cascading into the q3 AG chain.
        for pr in range(n_pair):
            filler.append((10 + pr, lambda pr=pr: yr_fetch(pr, 1)))
            filler.append((10 + pr, lambda pr=pr: yr_fetch(pr, 0)))
        for pr, g in ((0, 13), (1, 14), (2, 14)):
            filler.append((g, lambda pr=pr: yr_fetch(pr, 2)))
        # projection: h0 t-chunks fill late round 3 (the exp-bound
        # stretch with the least native PE work); q2/q3 drain in a
        # hand-ordered sequence: fetches whose AGs are long done, then
        # every ready matmul group (they overlap the final AG), then the
        # pr3 groups that must wait for it.
        for t in range(8):
            filler.append((13 if t < 4 else 14, lambda t=t: proj_atom(t)))
        filler.append((16, lambda: yr_fetch(3, 2)))
        for pr in range(n_pair):
            filler.append((16, lambda pr=pr: yr_fetch(pr, 3)))
        for t in range(8, 16):
            filler.append((16, lambda t=t: proj_main(t)))
        for t in range(8, 16):
            filler.append((16, lambda t=t: proj_tail(t)))

        def pop_filler(pos):
            for idx, (mp, thunk) in enumerate(filler):
                if mp <= pos:
                    filler.pop(idx)
                    thunk()
                    return True
            return False

        # ---- attention tile ------------------------------------------
        def att_tile(pr, j, pos):
            """Both heads of pair pr on q-tile j; 2-deep st -> exp -> av."""
            kT = qk_sb[n_pair + pr]
            qTt = qk_sb[pr]
            yps = {rr: ps.tile([P, QT], F32, tag=f"yp{rr}", bufs=1,
                               name=f"yp{rr}") for rr in range(2)}
            imax = KPQ * j + KPQ
            if pos > 0:
                pop_filler(pos)
                pop_filler(pos)
                pop_filler(pos)
            pend = []   # pipelined (i, off, pt) awaiting av

            def av(iv, offv, ptv):
                for rr in range(2):
                    nc.tensor.matmul(
                        yps[rr][0:HS + 1, offv:QT],
                        v_sb[iv][:, 2 * pr + rr, 0:HS + 1],
                        ptv[:, rr, offv:QT],
                        start=(iv == 0), stop=(iv == imax - 1))

            for i in range(imax):
                diag = (i // KPQ == j)
                # causally trim diagonal chunks to q >= i*KC
                off = KC * (i % KPQ) if diag else 0
                st = ps.tile([P, 2, QT], F32, tag="st", bufs=2, name="st")
                for rr in range(2):
                    ro = HS * rr
                    nc.tensor.matmul(
                        st[:, rr, off:QT],
                        kT[ro:ro + HS, i * KC:(i + 1) * KC],
                        qTt[ro:ro + HS, j * QT + off:(j + 1) * QT],
                        start=True, stop=True)
                pt = att.tile([P, 2, QT], BF16, tag="pt", bufs=4, name="pt")
                nc.scalar.activation(
                    pt[:, :, off:QT], st[:, :, off:QT],
                    mybir.ActivationFunctionType.Exp, scale=scale)
                if diag:
                    for rr in range(2):
                        # zero above the diagonal in the leading 128x128
                        # triangle, in place
                        nc.vector.tensor_mul(
                            pt[:, rr, off:off + KC],
                            pt[:, rr, off:off + KC], tri[:])
                pend.append((i, off, pt))
                if len(pend) > 2:
                    av(*pend.pop(0))
                if i % 2 == 1 or pos == 0:
                    pop_filler(pos)
            while pend:
                av(*pend.pop(0))

            # normalize both heads into one y2 tile (rr=1 write is
            # partition-shifted 0->64; validated on HW)
            y2 = att.tile([P, QT], BF16, tag="y2", bufs=3, name="y2")
            for rr in range(2):
                # keep this chain at exactly these 4 ops (see docstring)
                row = att.tile([1, QT], F32, tag="row", bufs=3, name="row")
                nc.vector.tensor_copy(row[:], yps[rr][HS:HS + 1, :])
                rec = att.tile([1, QT], F32, tag="rec", bufs=3, name="rec")
                nc.vector.reciprocal_approx_fast(rec[:], row[:])
                rb = att.tile([HS, QT], F32, tag="rb", bufs=3, name="rb")
                nc.gpsimd.partition_broadcast(rb[:], rec[:])
                nc.vector.tensor_mul(y2[rr * HS:(rr + 1) * HS, :],
                                     yps[rr][0:HS, :], rb[:])
            ti, to, col_base, j_set = segs[(pr, j)]
            nc.sync.dma_start(ti[:, col_base:col_base + QT], y2[:])
            if j == max(j_set):
                nc.gpsimd.collective_compute(
                    "AllGather", mybir.AluOpType.bypass,
                    replica_groups=PAIRS,
                    ins=[ti.opt()], outs=[to.opt()])

        # ---- schedule ------------------------------------------------
        # minimal upfront: what round 0 pair 0 needs; everything else
        # flows in through the filler queue
        qk_atom(0, 0)
        qk_atom(n_jt, 0)
        for t in range(4):
            v_atom(t)

        for j in range(n_jt):
            for pr in range(n_pair):
                att_tile(pr, j, j * n_pair + pr)
        while pop_filler(4 * n_jt):
            pass

    nc.compile()
    return nc


def shard_inputs(x, w_attn, b_attn, w_proj, b_proj):
    """Slice/transpose/shuffle full inputs into 8 per-core input maps."""
    Bq, T, C = x.shape
    CL = C // 2
    n_cc = C // P
    n_f = 2 * CL // P
    bf = ml_dtypes.bfloat16
    in_maps = []
    for i in range(N_CORES):
        b, g = i // 2, i % 2
        sl = slice(CL * g, CL * (g + 1))
        wq = w_attn[:, sl]
        wk = w_attn[:, C + CL * g:C + CL * (g + 1)]
        wvv = w_attn[:, 2 * C + CL * g:2 * C + CL * (g + 1)]
        wqk = np.concatenate([wq, wk], axis=1)          # [C, 2CL]
        # [C, 2CL] -> [f, p, c, m]: row r = c*128+p, col = f*128+m
        wqk_s = np.ascontiguousarray(
            wqk.reshape(n_cc, P, n_f, P).transpose(2, 1, 0, 3)).astype(bf)
        wv_shuf = np.ascontiguousarray(
            wvv.reshape(n_cc, P, CL).transpose(1, 0, 2)).astype(bf)
        wp_shuf = np.ascontiguousarray(
            w_proj[:, sl].reshape(n_cc, P, CL).transpose(1, 0, 2)).astype(bf)
        in_maps.append({
            "xT": np.ascontiguousarray(x[b].T).astype(bf),
            "wqk": wqk_s,
            "wv_s": wv_shuf,
            "wp_s": wp_shuf,
            "bqk": np.ascontiguousarray(
                np.concatenate([b_attn[sl],
                                b_attn[C + CL * g:C + CL * (g + 1)]])
                .reshape(n_f, P).T),
            "bv": np.ascontiguousarray(b_attn[2 * C + CL * g:2 * C + CL * (g + 1)]),
            "bp": np.ascontiguousarray(b_proj[sl]),
        })
    return in_maps


def gather_outputs(results, B, T, C):
    CL = C // 2
    out = np.empty((B, T, C), dtype=np.float32)
    for i in range(N_CORES):
        b, g = i // 2, i % 2
        out[b, :, CL * g:CL * (g + 1)] = results[i]["out"]
    return out


_NC_CACHE = {}


def get_nc(T, C):
    key = (T, C)
    if key not in _NC_CACHE:
        _NC_CACHE[key] = build_nc(T=T, C=C, HL=C // HS // 2)
    return _NC_CACHE[key]


def kernel(x, w_attn, b_attn, w_proj, b_proj):
    x = np.asarray(x, dtype=np.float32)
    w_attn = np.asarray(w_attn, dtype=np.float32)
    b_attn = np.asarray(b_attn, dtype=np.float32)
    w_proj = np.asarray(w_proj, dtype=np.float32)
    b_proj = np.asarray(b_proj, dtype=np.float32)

    Bq, T, C = x.shape
    nc = get_nc(T, C)

    in_maps = shard_inputs(x, w_attn, b_attn, w_proj, b_proj)
    trace = os.environ.get("KERNEL_TRACE", "0") == "1"
    res = bass_utils.run_bass_kernel_spmd(
        nc, in_maps, core_ids=list(range(N_CORES)), trace=trace)
    if trace and res.exec_time_ns is not None:
        print(f"HW exec time: {res.exec_time_ns} ns", flush=True)
        kernel.last_exec_time_ns = res.exec_time_ns
        kernel.last_results = res
    return gather_outputs(res.results, Bq, T, C)
